# revision 25
# baseline (speedup 1.0000x reference)
"""BinaryLSTM (binary tree-LSTM cell) Trainium2 kernel.

Full-input contract: kernel(**inputs) takes the complete unsharded tensors and
returns (h, c), each [8192, 1024] float32, matching the reference.

Strategy
--------
Data-parallel over the batch dim: core r handles rows r*1024:(r+1)*1024.
The 14 weight matrices are fused on the host into per-gate blocks over the
concatenated input X = [p | hl | hr] ([B, 3072]).  Gate pre-activations are
computed as z[h, b] so the contraction dim sits on SBUF partitions:

  z_g[h, b] = sum_k Vg[k, h] * XT[k, b]   (lhsT = Vg tile, rhs = XT tile)

PSUM tiles are [h_part=128, b_free=512] and the per-gate bias (varying along
h) is a per-partition [128,1] bias fused into the ACT sigmoid/tanh.

Matmuls run in float16 (full PE rate, ~216ns per 128x128x512 MM vs ~227ns
for float32r; quantization error ~8e-4 on h, measured in simulation).  PSUM
accumulation and everything downstream is fp32.

The kernel is PE-bound: steady-state MM pitch is 215.8ns = 512+6 cycles at
2.4GHz, the hardware floor for N=512 fp32-PSUM matmuls (N=1024 would cross
a PSUM bank, which a single matmul cannot).  Ramp mitigations (the engine
queues only start executing user instructions at ~6-7.4us after a fixed
framework preamble of barriers + register loads, and the first DMA lands
~11us in: 7.4us sync-queue preamble + 0.65us/dma_start issue + ~2us fixed
DMA latency + transfer):
  * NWARM dummy warm-up matmuls on a gpsimd-zeroed SBUF tile start at
    ~6.5us with no DMA dependency and bridge until the first real gemm's
    data lands (~14us).  This keeps the HAM clock-gate ramp (PE starts at
    ~4/8 clock, reaches 8/8 after ~4us of busy time) overlapped with the
    DMA pipeline fill, and avoids PE-idle gaps >~2us, which re-throttle
    the clock to 4/8 for 3.4us.
  * Early DMAs are 256-512KB, issued in strict first-use order (pf-weights
    m=0, xt[0], pf-weights m=1, xt[1..2], ...).  Concurrent transfers
    share the ~358GB/s HBM port fairly, so a big out-of-order transfer
    delays everything behind it (measured: a 2MB block issued 3rd starved
    the pf gemms for 8.5us and re-triggered the clock gate).
  * The pf phase runs m-PAIRS with k outermost, so each newly-arriving xt
    k-tile feeds 4 matmuls (~0.86us) — matching the ~0.7us/tile DMA
    delivery rate — instead of 2 (m-outer lost ~3us to arrival stalls).

The shared forget p-projection (pf = p @ Wf.T) is computed once per tile
(K=1024 accumulation), copied PSUM->SBUF (fp16), and added to the two
forget-gate child projections on DVE.  The i and u gates are computed
jointly via one Strassen level on their [3072, 2048] weight block (M split
at the i/u gate boundary); the o gate gets the same treatment with M split
at 512, pairing output tiles (m, m+4) for m=0..2 — m=3 and m=7 keep plain
o gemms so the kernel tail keeps its n_outer eviction overlap.  Both reuse
the SAME host-built moving-side combos (cb1..cb5); o-gate values for m+4
spill to DRAM scratch (fp16) and reload ~180us later.  Partial products
evict to fp16 SBUF via ACT and recombine on DVE; combine order matters:
the o-gate copies must come AFTER the i/u combines (shared mi rings,
bufs=1) and every weight ring must load in gemm-consumption order or the
scheduler deadlocks.  Total MACs: 12.3*B*D*H vs the 14*B*D*H dense
minimum (~48 matmuls saved per core).  Intermediates that tolerate fp16
(pre-activations, gate values, cl/cr, tanh(c)) are fp16 to fit the
Strassen combo/partial tiles in SBUF; measured rel err 4.6e-3 vs the
2e-2 gate.  The final o-tile eviction is chunked so ACT/DVE pipeline on
the tail.
"""

import os
import sys

for _p in ("/opt/trn_rl_repo", "/root/.axon_site/_ro/trn_rl_repo"):
    if os.path.isdir(_p) and _p not in sys.path:
        sys.path.append(_p)

import numpy as np

import concourse.bass as bass
import concourse.tile as tile
import concourse.mybir as mybir
from concourse import bacc
from concourse import bass_utils

B, D, H = 8192, 1024, 1024
NCORES = 8
BL = B // NCORES            # 1024 batch rows per core
K3 = 3 * D                  # 3072 contraction (p | hl | hr)
KT = K3 // 128              # 24 k-tiles
MT = H // 128               # 8 h-tiles (PSUM partition dim)
NFREE = 512                 # moving free dim per matmul (one PSUM bank, fp32)
NT = BL // NFREE            # 2 b-tiles
KC = 8                      # weight-chunk k-tiles per SBUF weight tile
NWARM = 18                  # clock-warm-up dummy matmuls

F32 = mybir.dt.float32
F16 = mybir.dt.float16

_CACHE = {}

# Results of the most recent hardware run (for test harness introspection).
LAST_RESULTS = None

# weight blocks: (name, #k-tiles, xt k-tile offset)
# i and u are computed jointly via one Strassen level on the [K=3072,
# M=2048] i|u weight block (K split at 1536, M split at 1024 = the i/u
# gate boundary, N split at 512 = the existing n=0/1 split), so each
# 128-row h-block m gets z_i and z_u from 7 multiplies of 12 k-tiles
# (84 MMs) instead of 2 gates x 24 k x 2 n (96 MMs).
KS = 12                    # strassen multiply contraction k-tiles (K/2)
WKINDS = {
    "o": (KT, 0),      # output gate, full K
    "pf": (D // 128, 0),              # shared forget p-projection (p rows)
    "fl": (2 * D // 128, D // 128),   # f_left child projections (hl|hr rows)
    "fr": (2 * D // 128, D // 128),   # f_right child projections
}
for _s in range(1, 8):
    WKINDS[f"s{_s}"] = (KS, 0)
# bias column index per activated gate
BIAS_IDX = {"i": 0, "fl": 1, "fr": 2, "u": 3, "o": 4}
# strassen multiply -> moving operand: ("cb", j) = host combo j, ("x", 0) =
# B11 = xt k 0..11 cols n=0, ("x", 1) = B22 = xt k 12..23 cols n=1
S_RHS = {
    1: ("cb", 1),   # M1 = (A11+A22)^T (B11+B22)
    2: ("x", 0),    # M2 = (A12+A22)^T B11
    3: ("cb", 2),   # M3 = A11^T (B12-B22)
    4: ("cb", 3),   # M4 = A22^T (B21-B11)
    5: ("x", 1),    # M5 = (A11+A21)^T B22
    6: ("cb", 4),   # M6 = (A12-A11)^T (B11+B12)
    7: ("cb", 5),   # M7 = (A21-A22)^T (B21+B22)
}


def _build_program():
    nc = bacc.Bacc("TRN2", target_bir_lowering=False, debug=False,
                   num_devices=NCORES)

    xt_d = nc.dram_tensor("xt", [K3, BL], F16, kind="ExternalInput").ap()
    w_d = {}
    for kind, (nk, _) in WKINDS.items():
        w_d[kind] = nc.dram_tensor(f"w_{kind}", [MT, 128, nk * 128], F16,
                                   kind="ExternalInput").ap()
    cb_d = {j: nc.dram_tensor(f"cb{j}", [KS, 128, NFREE], F16,
                              kind="ExternalInput").ap()
            for j in range(1, 6)}
    # o-gate strassen weights: M-half is 512 (4 m-tiles); pair groups
    # (j, j+4) for j=0..2 — m=3 and m=7 keep plain o gemms so the tail
    # keeps its n_outer overlap structure.
    so_d = {s: nc.dram_tensor(f"so{s}", [4, 128, KS * 128], F16,
                              kind="ExternalInput").ap()
            for s in range(1, 8)}
    gsp_d = nc.dram_tensor("gsp", [MT, 128, BL], F16, kind="Internal").ap()
    clt_d = nc.dram_tensor("clt", [H, BL], F16, kind="ExternalInput").ap()
    crt_d = nc.dram_tensor("crt", [H, BL], F16, kind="ExternalInput").ap()
    bt_d = nc.dram_tensor("bt", [128, 5 * MT], F32, kind="ExternalInput").ap()
    ht_d = nc.dram_tensor("ht", [H, BL], F32, kind="ExternalOutput").ap()
    ct_d = nc.dram_tensor("ct", [H, BL], F32, kind="ExternalOutput").ap()

    SIG = mybir.ActivationFunctionType.Sigmoid
    TANH = mybir.ActivationFunctionType.Tanh

    with tile.TileContext(nc) as tc:
        with tc.tile_pool(name="const", bufs=1) as const_pool, \
             tc.tile_pool(name="xtp", bufs=KT) as xt_pool, \
             tc.tile_pool(name="wp", bufs=8) as w_pool, \
             tc.tile_pool(name="gp", bufs=1) as g_pool, \
             tc.tile_pool(name="ep", bufs=2) as e_pool, \
             tc.tile_pool(name="pp", bufs=8, space="PSUM") as p_pool:

            # ---- PE clock warm-up: dummy matmuls with no DMA dependency ----
            # gpsimd memset: the gpsimd queue clears its framework preamble
            # first (~5.9us), so the warm matmuls start ~1.5us earlier than
            # with a DVE memset.
            warm = const_pool.tile([128, NFREE], F16, name="warm", tag="warm")
            nc.gpsimd.memset(warm[:], 0.0)
            ps_warm = p_pool.tile([128, NFREE], F32, name="ps_warm", tag="ps")
            for wi in range(NWARM):
                nc.tensor.matmul(ps_warm[:], warm[:, 0:128], warm[:],
                                 start=(wi == 0), stop=(wi == NWARM - 1))

            xt_r = xt_d.rearrange("(k p) b -> p k b", p=128)
            # xt k-tiles load as [k0 solo, 11 pairs, k23 solo] so the pf
            # phase's first dependency is small (256KB) but later tiles
            # arrive via half as many dma_start issues (each costs ~0.65us
            # of sync-queue time plus ~2us fixed DMA latency; per-tile
            # loads left k=3..7 arriving ~1us after the pf gemms wanted
            # them).
            xpair = [None] * 11
            xsolo = {}

            def load_x(k):
                """Load xt tile k (k=0,23) or the pair (k, k+1) for odd k."""
                if k in (0, KT - 1):
                    t = xt_pool.tile([128, BL], F16, name=f"x_{k}", tag="x",
                                     bufs=2)
                    nc.sync.dma_start(t[:], xt_r[:, k, :])
                    xsolo[k] = t
                else:
                    assert k % 2 == 1
                    t = xt_pool.tile([128, 2, BL], F16, name=f"x_{k}",
                                     tag="x2", bufs=11)
                    nc.sync.dma_start(t[:], xt_r[:, k:k + 2, :])
                    xpair[(k - 1) // 2] = t

            def xt_ap(k, n):
                """[128, NFREE] view of xt k-tile k, b-chunk n."""
                sf = slice(n * NFREE, (n + 1) * NFREE)
                if k in (0, KT - 1):
                    return xsolo[k][:, sf]
                return xpair[(k - 1) // 2][:, (k - 1) % 2, sf]

            def load_w(kind, m, pool=None, tag="w", eng=None):
                """Load the weight block for (kind, m) as KC-k-tile chunks."""
                nk, _ = WKINDS[kind]
                chunks = []
                for c in range(0, nk, KC):
                    nkc = min(KC, nk - c)
                    t = (pool or w_pool).tile([128, nkc, 128], F16,
                                              name=f"w_{kind}_{m}_{c}",
                                              tag=tag)
                    (eng or nc.sync).dma_start(
                        t[:],
                        w_d[kind][m][:, c * 128:(c + nkc) * 128]
                        .rearrange("p (k c) -> p k c", k=nkc))
                    chunks.append(t)
                return chunks

            # Early DMAs in strict first-use order, all <=256KB.  pf weights
            # live in const_pool (per-m tags) so they never contend with the
            # main-phase weight ring.  (Tried issuing the first two loads on
            # the gpsimd queue, which wakes ~1.4us earlier — but its SWDGE
            # software descriptor generation made them land ~2us LATER than
            # sync-queue HWDGE issues.)
            w_pfs = [None] * MT
            w_pfs[0] = load_w("pf", 0, pool=const_pool, tag="wpf0")
            load_x(0)
            w_pfs[1] = load_w("pf", 1, pool=const_pool, tag="wpf1")
            load_x(1)
            load_x(3)
            w_pfs[2] = load_w("pf", 2, pool=const_pool, tag="wpf2")
            w_pfs[3] = load_w("pf", 3, pool=const_pool, tag="wpf3")
            load_x(5)
            load_x(7)
            for m in range(4, MT):
                w_pfs[m] = load_w("pf", m, pool=const_pool, tag=f"wpf{m}")

            bt_t = const_pool.tile([128, 5 * MT], F32, name="bt_t", tag="bt")
            nc.sync.dma_start(bt_t[:], bt_d)

            # m=0 forget-gate weights next (first thing the main phase
            # needs), then the rest of XT, then the host-built Strassen
            # moving-side combos (first used by m=0's i/u multiplies at
            # ~55us).
            w_fl0 = load_w("fl", 0)
            w_fr0 = load_w("fr", 0)
            cb_t = {}

            def load_cb(j):
                t = const_pool.tile([128, KS, NFREE], F16, name=f"cb{j}",
                                    tag=f"cb{j}")
                nc.sync.dma_start(t[:], cb_d[j].rearrange("k p n -> p k n"))
                cb_t[j] = t

            def load_ws(s, m):
                t = w_pool.tile([128, KS, 128], F16,
                                name=f"w_s{s}_{m}", tag="ws", bufs=3)
                nc.sync.dma_start(
                    t[:],
                    w_d[f"s{s}"][m].rearrange("p (k c) -> p k c", k=KS))
                return t

            def load_wso(s, j):
                t = w_pool.tile([128, KS, 128], F16,
                                name=f"w_so{s}_{j}", tag="ws", bufs=3)
                nc.sync.dma_start(
                    t[:],
                    so_d[s][j].rearrange("p (k c) -> p k c", k=KS))
                return t

            # Phase O's first strassen weight blocks go out BEFORE the
            # 7.5MB of cb combos so they aren't bandwidth-starved when the
            # o-strassen phase starts (~45us).  Exactly ring-capacity tiles
            # are preloaded so no issue blocks the sync queue on a gemm WAR.
            w_s0 = {}
            for k in (9, 11, 13, 15):
                load_x(k)
            for k in (17, 19, 21, 23):
                load_x(k)
            w_s0[2] = load_ws(2, 0)
            w_s0[5] = load_ws(5, 0)
            load_cb(1)
            w_s0[1] = load_ws(1, 0)
            load_cb(2)
            for j in (3, 4, 5):
                load_cb(j)

            def s_rhs(s, k):
                """Moving operand for strassen multiply s, k-tile k."""
                kind, arg = S_RHS[s]
                if kind == "cb":
                    return cb_t[arg][:, k, :]
                if arg == 0:
                    return xt_ap(k, 0)          # B11
                return xt_ap(KS + k, 1)         # B22

            def pf_w(m, k):
                return w_pfs[m][0][:, k, :]

            def gemm(kind, m, w_chunks, n_outer=False):
                """Accumulate the gate block, return NT psum tiles.

                n_outer=True finishes all of n=0 before starting n=1 so
                n=0's eviction chain overlaps n=1's matmuls (tail shave
                for the final gate).
                """
                nk, koff = WKINDS[kind]
                ps = [p_pool.tile([128, NFREE], F32,
                                  name=f"ps_{kind}_{m}_{n}", tag="ps")
                      for n in range(NT)]
                if n_outer:
                    for n in range(NT):
                        for k in range(nk):
                            nc.tensor.matmul(
                                ps[n][:], w_chunks[k // KC][:, k % KC, :],
                                xt_ap(koff + k, n),
                                start=(k == 0), stop=(k == nk - 1))
                else:
                    for k in range(nk):
                        w_t = w_chunks[k // KC]
                        for n in range(NT):
                            nc.tensor.matmul(
                                ps[n][:], w_t[:, k % KC, :],
                                xt_ap(koff + k, n),
                                start=(k == 0), stop=(k == nk - 1))
                return ps

            # Phase 1: all pf gemms (only consume XT k-tiles 0..7).  Their
            # SBUF results stay resident until each m's forget gates run.
            # m-pairs with k outermost: each new xt k-tile feeds 4 MMs
            # (~0.86us at full clock), matching the ~0.7us/tile DMA
            # delivery rate during the ramp, so the pf phase never stalls
            # on xt arrivals (m-outer consumed a tile per 0.43us and lost
            # ~3us to stalls + HAM clock re-gating).
            pf_sbs = [None] * MT
            for m0 in range(0, MT, 2):
                pair = (m0, m0 + 1)
                ps_pf = {}
                for m in pair:
                    for n in range(NT):
                        ps_pf[(m, n)] = p_pool.tile(
                            [128, NFREE], F32,
                            name=f"ps_pf_{m}_{n}", tag="ps")
                for k in range(KC):
                    for m in pair:
                        for n in range(NT):
                            nc.tensor.matmul(
                                ps_pf[(m, n)][:], pf_w(m, k), xt_ap(k, n),
                                start=(k == 0), stop=(k == KC - 1))
                for m in pair:
                    pf_sb = []
                    for n in range(NT):
                        t = g_pool.tile([128, NFREE], F16,
                                        name=f"pf_{m}_{n}", tag="pf",
                                        bufs=MT * NT)
                        nc.scalar.copy(t[:], ps_pf[(m, n)][:])
                        pf_sb.append(t)
                    pf_sbs[m] = pf_sb

            # combination term tables: for a strassen M-half pair (A, B),
            # per half and n-chunk: (first two signed Mi, optional last two)
            S_TERMS = ((((1, 4), (-5, 7)), ((3, 5), None)),
                       (((2, 4), None), ((1, -2), (3, 6))))

            def s_combine(mi, half, n):
                pos, extra = S_TERMS[half][n]
                z = e_pool.tile([128, NFREE], F16,
                                name=f"zc_{half}_{n}", tag="zs", bufs=2)
                a, b = pos
                if b < 0:
                    nc.vector.tensor_sub(z[:], mi[a][:], mi[-b][:])
                else:
                    nc.vector.tensor_add(z[:], mi[a][:], mi[b][:])
                if extra is not None:
                    for t_ in extra:
                        if t_ < 0:
                            nc.vector.tensor_sub(z[:], z[:], mi[-t_][:])
                        else:
                            nc.vector.tensor_add(z[:], z[:], mi[t_][:])
                return z

            for m in range(MT):
                pf_sb = pf_sbs[m]
                w_fl = w_fl0 if m == 0 else load_w("fl", m)
                w_fr = w_fr0 if m == 0 else load_w("fr", m)
                # strassen weights as single 12-k-tile chunks in their own
                # ring: fewer/bigger DMAs with deeper prefetch (21 small
                # chunks/m against an 8-slot ring serialized issues and
                # left one group-start LDW waiting ~2.6us on its weights
                # every m-tile)
                # load in gemm-consumption order (2,5,1,3,4,6,7): with a
                # 3-slot ring, loading s1..s7 in numeric order deadlocks
                # (s4's issue blocks the sync FIFO on s1's gemm, which is
                # queued after s5's gemm, whose weights sit behind s4)
                w_s = {}
                for s in (2, 5, 1, 3, 4, 6, 7):
                    if m == 0 and s in w_s0:
                        w_s[s] = [w_s0[s]]
                    else:
                        w_s[s] = [load_ws(s, m)]
                w_so = {}
                if m < 3:
                    for s in (2, 5, 1, 3, 4, 6, 7):
                        w_so[s] = load_wso(s, m)
                plain_o = m in (3, MT - 1)
                if plain_o:
                    w_o = load_w("o", m)
                elif m >= 4:
                    gor = e_pool.tile([128, BL], F16, name=f"gor_{m}",
                                      tag="gor", bufs=1)
                    nc.sync.dma_start(gor[:], gsp_d[m])

                gates = {}
                zf_tiles = {}
                for kind, w_t in (("fl", w_fl), ("fr", w_fr)):
                    ps = gemm(kind, m, w_t)
                    for n in range(NT):
                        # the DVE add evicts the PSUM tile immediately; the
                        # sigmoid is deferred until after the strassen
                        # section so the mi PSUM-evictions sit at the HEAD
                        # of the ACT queue (a WAR-blocked matmul waiting on
                        # an ACT eviction queued behind gate sigmoids cost
                        # ~2.6us per m-tile)
                        z = e_pool.tile([128, NFREE], F16,
                                        name=f"z_{kind}_{m}_{n}",
                                        tag="zf", bufs=4)
                        nc.vector.tensor_add(z[:], ps[n][:], pf_sb[n][:])
                        zf_tiles[(kind, n)] = z

                # i and u via one Strassen level: 7 multiplies of KS=12
                # k-tiles each, partial products evicted to fp16 SBUF and
                # combined on DVE into the 4 pre-activations
                # (zi_n0=M1+M4-M5+M7, zi_n1=M3+M5, zu_n0=M2+M4,
                #  zu_n1=M1-M2+M3+M6).
                mi = {}
                for s in (2, 5, 1, 3, 4, 6, 7):
                    ps_s = p_pool.tile([128, NFREE], F32,
                                       name=f"ps_s{s}_{m}", tag="ps")
                    wch = w_s[s][0]
                    for k in range(KS):
                        nc.tensor.matmul(
                            ps_s[:], wch[:, k, :], s_rhs(s, k),
                            start=(k == 0), stop=(k == KS - 1))
                    t = g_pool.tile([128, NFREE], F16, name=f"mi_{s}_{m}",
                                    tag=f"mi{s}", bufs=1)
                    nc.scalar.copy(t[:], ps_s[:])
                    mi[s] = t

                # deferred forget-gate activations (ACT, after mi copies)
                for kind in ("fl", "fr"):
                    bi = BIAS_IDX[kind]
                    for n in range(NT):
                        gt = g_pool.tile([128, NFREE], F16,
                                         name=f"g_{kind}_{m}_{n}",
                                         tag=f"g{kind}", bufs=2)
                        nc.scalar.activation(
                            gt[:], zf_tiles[(kind, n)][:], SIG,
                            bias=bt_t[:, bi * MT + m: bi * MT + m + 1])
                        gates[(kind, n)] = gt

                for gate, act, bi, terms in (
                        ("i", SIG, 0, (((1, 4), (-5, 7)), ((3, 5), None))),
                        ("u", TANH, 3, (((2, 4), None), ((1, -2), (3, 6))))):
                    for n in range(NT):
                        pos, extra = terms[n]
                        z = e_pool.tile([128, NFREE], F16,
                                        name=f"z_{gate}_{m}_{n}", tag="zs",
                                        bufs=2)
                        a, b = pos
                        if b < 0:
                            nc.vector.tensor_sub(z[:], mi[a][:], mi[-b][:])
                        else:
                            nc.vector.tensor_add(z[:], mi[a][:], mi[b][:])
                        if extra is not None:
                            c1_, c2_ = extra
                            if c1_ < 0:
                                nc.vector.tensor_sub(z[:], z[:], mi[-c1_][:])
                            else:
                                nc.vector.tensor_add(z[:], z[:], mi[c1_][:])
                            if c2_ < 0:
                                nc.vector.tensor_sub(z[:], z[:], mi[-c2_][:])
                            else:
                                nc.vector.tensor_add(z[:], z[:], mi[c2_][:])
                        gt = g_pool.tile([128, NFREE], F16,
                                         name=f"g_{gate}_{m}_{n}",
                                         tag=f"g{gate}", bufs=2)
                        nc.scalar.activation(
                            gt[:], z[:], act,
                            bias=bt_t[:, bi * MT + m: bi * MT + m + 1])
                        gates[(gate, n)] = gt

                go_sb = {}
                if m < 3:
                    # o-gate strassen for the pair (m, m+4), interleaved
                    # into the m-loop so its weight traffic spreads across
                    # the iteration instead of colliding with the early
                    # cb/xt DMAs (a standalone phase-O stalled 22us).
                    mi_o = {}
                    for s_ in (2, 5, 1, 3, 4, 6, 7):
                        wso = w_so[s_]
                        ps_s = p_pool.tile([128, NFREE], F32,
                                           name=f"ps_so{s_}_{m}", tag="ps")
                        for k in range(KS):
                            nc.tensor.matmul(
                                ps_s[:], wso[:, k, :], s_rhs(s_, k),
                                start=(k == 0), stop=(k == KS - 1))
                        t = g_pool.tile([128, NFREE], F16,
                                        name=f"mo_{s_}_{m}",
                                        tag=f"mi{s_}", bufs=1)
                        nc.scalar.copy(t[:], ps_s[:])
                        mi_o[s_] = t
                    for half, mm in ((0, m), (1, m + 4)):
                        for n in range(NT):
                            z = s_combine(mi_o, half, n)
                            go = e_pool.tile([128, NFREE], F16,
                                             name=f"go_{mm}_{n}", tag="go")
                            nc.scalar.activation(
                                go[:], z[:], SIG,
                                bias=bt_t[:, 4 * MT + mm: 4 * MT + mm + 1])
                            if half == 0:
                                go_sb[n] = go
                            else:
                                nc.sync.dma_start(
                                    gsp_d[mm][:, n * NFREE:(n + 1) * NFREE],
                                    go[:])

                # c-chain: independent of o, overlaps o's matmuls.  cl/cr
                # arrive fp16 and the fl*cl / fr*cr products overwrite them
                # in place (saves two fp32 scratch tags of SBUF).
                th_tiles = {}
                for n in range(NT):
                    sp = slice(m * 128, (m + 1) * 128)
                    sf = slice(n * NFREE, (n + 1) * NFREE)
                    cl_t = e_pool.tile([128, NFREE], F16,
                                       name=f"cl_{m}_{n}", tag="cl")
                    nc.sync.dma_start(cl_t[:], clt_d[sp, sf])
                    cr_t = e_pool.tile([128, NFREE], F16,
                                       name=f"cr_{m}_{n}", tag="cr")
                    nc.sync.dma_start(cr_t[:], crt_d[sp, sf])

                    iu = e_pool.tile([128, NFREE], F32,
                                     name=f"iu_{m}_{n}", tag="iu")
                    nc.vector.tensor_mul(iu[:], gates[("i", n)][:],
                                         gates[("u", n)][:])
                    nc.vector.tensor_mul(cl_t[:], gates[("fl", n)][:],
                                         cl_t[:])
                    nc.vector.tensor_mul(cr_t[:], gates[("fr", n)][:],
                                         cr_t[:])
                    # c accumulates in-place in iu
                    nc.vector.tensor_add(iu[:], iu[:], cl_t[:])
                    nc.vector.tensor_add(iu[:], iu[:], cr_t[:])
                    nc.sync.dma_start(ct_d[sp, sf], iu[:])

                    th = e_pool.tile([128, NFREE], F16,
                                     name=f"th_{m}_{n}", tag="th")
                    nc.scalar.activation(th[:], iu[:], TANH)
                    th_tiles[n] = th

                if not plain_o:
                    # o came from the in-loop strassen pair (m<3: SBUF;
                    # m>=4: DRAM spill reload)
                    for n in range(NT):
                        sp = slice(m * 128, (m + 1) * 128)
                        sf = slice(n * NFREE, (n + 1) * NFREE)
                        h_t = e_pool.tile([128, NFREE], F32,
                                          name=f"h_{m}_{n}", tag="h")
                        o_ap = go_sb[n][:] if m < 3 else gor[:, sf]
                        nc.vector.tensor_mul(h_t[:], o_ap, th_tiles[n][:])
                        nc.sync.dma_start(ht_d[sp, sf], h_t[:])
                    continue

                ps_o = gemm("o", m, w_o, n_outer=True)
                for n in range(NT):
                    sp = slice(m * 128, (m + 1) * 128)
                    sf = slice(n * NFREE, (n + 1) * NFREE)
                    # chunk the very last eviction's ACT/DVE so they
                    # pipeline on the kernel tail; keep a single DMA (a
                    # second dma_start costs ~0.6us of serialized sync-queue
                    # issue, more than the larger transfer)
                    last = (m == MT - 1 and n == NT - 1)
                    nch = 2 if last else 1
                    cw = NFREE // nch
                    go = e_pool.tile([128, NFREE], F16,
                                     name=f"g_o_{m}_{n}", tag="go")
                    h_t = e_pool.tile([128, NFREE], F32,
                                      name=f"h_{m}_{n}", tag="h")
                    for ci in range(nch):
                        cs = slice(ci * cw, (ci + 1) * cw)
                        nc.scalar.activation(
                            go[:, cs], ps_o[n][:, cs], SIG,
                            bias=bt_t[:, 4 * MT + m: 4 * MT + m + 1])
                        nc.vector.tensor_mul(h_t[:, cs], go[:, cs],
                                             th_tiles[n][:, cs])
                    nc.sync.dma_start(ht_d[sp, sf], h_t[:])

    nc.compile()
    return nc


def _get_program():
    if "nc" not in _CACHE:
        _CACHE["nc"] = _build_program()
    return _CACHE["nc"]


def _tile_weight(V, nk, mt=MT):
    """[nk*128, mt*128] -> [mt, 128, nk*128] with [m][kp, k*128+mc] = V[k*128+kp, m*128+mc]."""
    return np.ascontiguousarray(
        V.reshape(nk, 128, mt, 128)
         .transpose(2, 1, 0, 3)
         .reshape(mt, 128, nk * 128)
         .astype(np.float16))


def kernel(hl, cl, hr, cr, p,
           Wd, Wdl, Wdr, bd,
           Wf, Wfll, Wflr, Wfrl, Wfrr, bfl, bfr,
           Wo, Wol, Wor, bo,
           Wi, Wil, Wir, bi):
    global LAST_RESULTS
    f32 = np.float32
    hl, cl, hr, cr, p = (np.asarray(a, dtype=f32) for a in (hl, cl, hr, cr, p))
    ws = {k: np.asarray(v, dtype=f32) for k, v in dict(
        Wd=Wd, Wdl=Wdl, Wdr=Wdr, Wf=Wf, Wfll=Wfll, Wflr=Wflr, Wfrl=Wfrl,
        Wfrr=Wfrr, Wo=Wo, Wol=Wol, Wor=Wor, Wi=Wi, Wil=Wil, Wir=Wir).items()}

    # Wf{gate l/r}{child l/r}: f_left mixes hl via Wfll and hr via Wflr;
    # f_right mixes hl via Wfrl and hr via Wfrr.
    # i|u joint weight block [K=3072, M=2048] split into Strassen quadrants
    # (A11=[K1,i], A12=[K1,u], A21=[K2,i], A22=[K2,u]; K1 = first 1536 rows)
    # and combined on the host in fp32 before the fp16 cast.
    Wi_blk = np.concatenate([ws["Wd"].T, ws["Wdl"].T, ws["Wdr"].T], 0)
    Wu_blk = np.concatenate([ws["Wi"].T, ws["Wil"].T, ws["Wir"].T], 0)
    K1 = KS * 128
    A11, A21 = Wi_blk[:K1], Wi_blk[K1:]
    A12, A22 = Wu_blk[:K1], Wu_blk[K1:]
    s_w = {1: A11 + A22, 2: A12 + A22, 3: A11, 4: A22,
           5: A11 + A21, 6: A12 - A11, 7: A21 - A22}
    wt = {
        "o": _tile_weight(np.concatenate(
            [ws["Wo"].T, ws["Wol"].T, ws["Wor"].T], 0), KT),
        "pf": _tile_weight(np.ascontiguousarray(ws["Wf"].T), 8),
        "fl": _tile_weight(np.concatenate(
            [ws["Wfll"].T, ws["Wflr"].T], 0), 16),
        "fr": _tile_weight(np.concatenate(
            [ws["Wfrl"].T, ws["Wfrr"].T], 0), 16),
    }
    for s, Vw in s_w.items():
        wt[f"s{s}"] = _tile_weight(np.ascontiguousarray(Vw), KS)
    # o-gate strassen quadrants: M-half = 512 (4 m-tiles)
    Wo_blk = np.concatenate([ws["Wo"].T, ws["Wol"].T, ws["Wor"].T], 0)
    O11, O12 = Wo_blk[:K1, :512], Wo_blk[:K1, 512:]
    O21, O22 = Wo_blk[K1:, :512], Wo_blk[K1:, 512:]
    so_w = {1: O11 + O22, 2: O12 + O22, 3: O11, 4: O22,
            5: O11 + O21, 6: O12 - O11, 7: O21 - O22}
    so_t = {s: _tile_weight(np.ascontiguousarray(Vw), KS, mt=4)
            for s, Vw in so_w.items()}

    Bt = np.empty((128, 5 * MT), dtype=f32)
    for name, b_ in (("i", bd), ("fl", bfl), ("fr", bfr), ("u", bi), ("o", bo)):
        gi = BIAS_IDX[name]
        Bt[:, gi * MT:(gi + 1) * MT] = np.asarray(b_, dtype=f32).reshape(MT, 128).T

    X = np.concatenate([p, hl, hr], axis=1)    # [B, 3D]

    f16 = np.float16
    in_maps = []
    for r in range(NCORES):
        rows = slice(r * BL, (r + 1) * BL)
        XT = X[rows].T.astype(f16)                    # [3D, BL] fp16
        XTf = XT.astype(f32)
        N1 = NFREE
        B11, B12 = XTf[:K1, :N1], XTf[:K1, N1:]
        B21, B22 = XTf[K1:, :N1], XTf[K1:, N1:]
        cbs = {1: B11 + B22, 2: B12 - B22, 3: B21 - B11,
               4: B11 + B12, 5: B21 + B22}
        im = {
            "xt": np.ascontiguousarray(XT),
            "clt": np.ascontiguousarray(cl[rows].T.astype(f16)),
            "crt": np.ascontiguousarray(cr[rows].T.astype(f16)),
            "bt": Bt,
        }
        for j, cb in cbs.items():
            im[f"cb{j}"] = np.ascontiguousarray(
                cb.astype(f16).reshape(KS, 128, NFREE))
        for kind, arr in wt.items():
            im[f"w_{kind}"] = arr
        for s, arr in so_t.items():
            im[f"so{s}"] = arr
        in_maps.append(im)

    nc = _get_program()
    res = bass_utils.run_bass_kernel_spmd(nc, in_maps,
                                          core_ids=list(range(NCORES)))
    LAST_RESULTS = res

    h = np.empty((B, H), dtype=f32)
    c = np.empty((B, H), dtype=f32)
    for r in range(NCORES):
        rows = slice(r * BL, (r + 1) * BL)
        h[rows] = res.results[r]["ht"].T
        c[rows] = res.results[r]["ct"].T
    return (h, c)


# revision 26
# speedup vs baseline: 1.1850x; 1.1850x over previous
"""BinaryLSTM (binary tree-LSTM cell) Trainium2 kernel.

Full-input contract: kernel(**inputs) takes the complete unsharded tensors and
returns (h, c), each [8192, 1024] float32, matching the reference.

Strategy
--------
Data-parallel over the batch dim: core r handles rows r*1024:(r+1)*1024.
The 14 weight matrices are fused on the host into per-gate blocks over the
concatenated input X = [p | hl | hr] ([B, 3072]).  Gate pre-activations are
computed as z[h, b] so the contraction dim sits on SBUF partitions:

  z_g[h, b] = sum_k Vg[k, h] * XT[k, b]   (lhsT = Vg tile, rhs = XT tile)

PSUM tiles are [h_part=128, b_free=512] and the per-gate bias (varying along
h) is a per-partition [128,1] bias fused into the ACT sigmoid/tanh.

Matmuls run in float16 (full PE rate, ~216ns per 128x128x512 MM vs ~227ns
for float32r; quantization error ~8e-4 on h, measured in simulation).  PSUM
accumulation and everything downstream is fp32.

The kernel is PE-bound: steady-state MM pitch is 215.8ns = 512+6 cycles at
2.4GHz, the hardware floor for N=512 fp32-PSUM matmuls (N=1024 would cross
a PSUM bank, which a single matmul cannot).  Ramp mitigations (the engine
queues only start executing user instructions at ~6-7.4us after a fixed
framework preamble of barriers + register loads, and the first DMA lands
~11us in: 7.4us sync-queue preamble + 0.65us/dma_start issue + ~2us fixed
DMA latency + transfer):
  * NWARM dummy warm-up matmuls on a gpsimd-zeroed SBUF tile start at
    ~6.5us with no DMA dependency and bridge until the first real gemm's
    data lands (~14us).  This keeps the HAM clock-gate ramp (PE starts at
    ~4/8 clock, reaches 8/8 after ~4us of busy time) overlapped with the
    DMA pipeline fill, and avoids PE-idle gaps >~2us, which re-throttle
    the clock to 4/8 for 3.4us.
  * Early DMAs are 256-512KB, issued in strict first-use order (pf-weights
    m=0, xt[0], pf-weights m=1, xt[1..2], ...).  Concurrent transfers
    share the ~358GB/s HBM port fairly, so a big out-of-order transfer
    delays everything behind it (measured: a 2MB block issued 3rd starved
    the pf gemms for 8.5us and re-triggered the clock gate).
  * The pf phase runs m-PAIRS with k outermost, so each newly-arriving xt
    k-tile feeds 4 matmuls (~0.86us) — matching the ~0.7us/tile DMA
    delivery rate — instead of 2 (m-outer lost ~3us to arrival stalls).

The shared forget p-projection (pf = p @ Wf.T) is computed once per tile
(K=1024 accumulation), copied PSUM->SBUF (fp16), and added to the two
forget-gate child projections on DVE.  The i and u gates are computed
jointly via one Strassen level on their [3072, 2048] weight block (M split
at the i/u gate boundary); the o gate gets the same treatment with M split
at 512, pairing output tiles (m, m+4) for m=0..2 — m=3 and m=7 keep plain
o gemms so the kernel tail keeps its n_outer eviction overlap.  Both reuse
the SAME host-built moving-side combos (cb1..cb5); o-gate values for m+4
spill to DRAM scratch (fp16) and reload ~180us later.  Partial products
evict to fp16 SBUF via ACT and recombine on DVE; combine order matters:
the o-gate copies must come AFTER the i/u combines (shared mi rings,
bufs=1) and every weight ring must load in gemm-consumption order or the
scheduler deadlocks.  Total MACs: 12.3*B*D*H vs the 14*B*D*H dense
minimum (~48 matmuls saved per core).  Intermediates that tolerate fp16
(pre-activations, gate values, cl/cr, tanh(c)) are fp16 to fit the
Strassen combo/partial tiles in SBUF; measured rel err 4.6e-3 vs the
2e-2 gate.  The final o-tile eviction is chunked so ACT/DVE pipeline on
the tail.
"""

import os
import sys

for _p in ("/opt/trn_rl_repo", "/root/.axon_site/_ro/trn_rl_repo"):
    if os.path.isdir(_p) and _p not in sys.path:
        sys.path.append(_p)

import numpy as np

import concourse.bass as bass
import concourse.tile as tile
import concourse.mybir as mybir
from concourse import bacc
from concourse import bass_utils

B, D, H = 8192, 1024, 1024
NCORES = 8
BL = B // NCORES            # 1024 batch rows per core
K3 = 3 * D                  # 3072 contraction (p | hl | hr)
KT = K3 // 128              # 24 k-tiles
MT = H // 128               # 8 h-tiles (PSUM partition dim)
NFREE = 512                 # moving free dim per matmul (one PSUM bank, fp32)
NT = BL // NFREE            # 2 b-tiles
KC = 8                      # weight-chunk k-tiles per SBUF weight tile
NWARM = 18                  # clock-warm-up dummy matmuls

F32 = mybir.dt.float32
F16 = mybir.dt.float16

_CACHE = {}

# Results of the most recent hardware run (for test harness introspection).
LAST_RESULTS = None

# weight blocks: (name, #k-tiles, xt k-tile offset)
# i and u are computed jointly via one Strassen level on the [K=3072,
# M=2048] i|u weight block (K split at 1536, M split at 1024 = the i/u
# gate boundary, N split at 512 = the existing n=0/1 split), so each
# 128-row h-block m gets z_i and z_u from 7 multiplies of 12 k-tiles
# (84 MMs) instead of 2 gates x 24 k x 2 n (96 MMs).
KS = 12                    # strassen multiply contraction k-tiles (K/2)
WKINDS = {
    "o": (KT, 0),      # output gate, full K
    "pf": (D // 128, 0),              # shared forget p-projection (p rows)
    "fl": (2 * D // 128, D // 128),   # f_left child projections (hl|hr rows)
    "fr": (2 * D // 128, D // 128),   # f_right child projections
}
for _s in range(1, 8):
    WKINDS[f"s{_s}"] = (KS, 0)
# bias column index per activated gate
BIAS_IDX = {"i": 0, "fl": 1, "fr": 2, "u": 3, "o": 4}
# strassen multiply -> moving operand: ("cb", j) = host combo j, ("x", 0) =
# B11 = xt k 0..11 cols n=0, ("x", 1) = B22 = xt k 12..23 cols n=1
S_RHS = {
    1: ("cb", 1),   # M1 = (A11+A22)^T (B11+B22)
    2: ("x", 0),    # M2 = (A12+A22)^T B11
    3: ("cb", 2),   # M3 = A11^T (B12-B22)
    4: ("cb", 3),   # M4 = A22^T (B21-B11)
    5: ("x", 1),    # M5 = (A11+A21)^T B22
    6: ("cb", 4),   # M6 = (A12-A11)^T (B11+B12)
    7: ("cb", 5),   # M7 = (A21-A22)^T (B21+B22)
}


def _build_program():
    nc = bacc.Bacc("TRN2", target_bir_lowering=False, debug=False,
                   num_devices=NCORES)

    xt_d = nc.dram_tensor("xt", [K3, BL], F16, kind="ExternalInput").ap()
    w_d = {}
    for kind, (nk, _) in WKINDS.items():
        w_d[kind] = nc.dram_tensor(f"w_{kind}", [MT, 128, nk * 128], F16,
                                   kind="ExternalInput").ap()
    cb_d = {j: nc.dram_tensor(f"cb{j}", [KS, 128, NFREE], F16,
                              kind="ExternalInput").ap()
            for j in range(1, 6)}
    # o-gate strassen weights: M-half is 512 (4 m-tiles); pair groups
    # (j, j+4) for j=0..2 — m=3 and m=7 keep plain o gemms so the tail
    # keeps its n_outer overlap structure.
    so_d = {s: nc.dram_tensor(f"so{s}", [4, 128, KS * 128], F16,
                              kind="ExternalInput").ap()
            for s in range(1, 8)}
    gsp_d = nc.dram_tensor("gsp", [MT, 128, BL], F16, kind="Internal").ap()
    clt_d = nc.dram_tensor("clt", [H, BL], F16, kind="ExternalInput").ap()
    crt_d = nc.dram_tensor("crt", [H, BL], F16, kind="ExternalInput").ap()
    bt_d = nc.dram_tensor("bt", [128, 5 * MT], F32, kind="ExternalInput").ap()
    ht_d = nc.dram_tensor("ht", [H, BL], F32, kind="ExternalOutput").ap()
    ct_d = nc.dram_tensor("ct", [H, BL], F32, kind="ExternalOutput").ap()

    SIG = mybir.ActivationFunctionType.Sigmoid
    TANH = mybir.ActivationFunctionType.Tanh

    with tile.TileContext(nc) as tc:
        with tc.tile_pool(name="const", bufs=1) as const_pool, \
             tc.tile_pool(name="xtp", bufs=KT) as xt_pool, \
             tc.tile_pool(name="wp", bufs=6) as w_pool, \
             tc.tile_pool(name="gp", bufs=1) as g_pool, \
             tc.tile_pool(name="ep", bufs=2) as e_pool, \
             tc.tile_pool(name="pp", bufs=8, space="PSUM") as p_pool:

            # ---- PE clock warm-up: dummy matmuls with no DMA dependency ----
            # gpsimd memset: the gpsimd queue clears its framework preamble
            # first (~5.9us), so the warm matmuls start ~1.5us earlier than
            # with a DVE memset.
            warm = const_pool.tile([128, NFREE], F16, name="warm", tag="warm")
            nc.gpsimd.memset(warm[:], 0.0)
            ps_warm = p_pool.tile([128, NFREE], F32, name="ps_warm", tag="ps")
            for wi in range(NWARM):
                nc.tensor.matmul(ps_warm[:], warm[:, 0:128], warm[:],
                                 start=(wi == 0), stop=(wi == NWARM - 1))

            xt_r = xt_d.rearrange("(k p) b -> p k b", p=128)
            # xt k-tiles load as [k0 solo, 11 pairs, k23 solo] so the pf
            # phase's first dependency is small (256KB) but later tiles
            # arrive via half as many dma_start issues (each costs ~0.65us
            # of sync-queue time plus ~2us fixed DMA latency; per-tile
            # loads left k=3..7 arriving ~1us after the pf gemms wanted
            # them).
            xpair = [None] * 11
            xsolo = {}

            def load_x(k):
                """Load xt tile k (k=0,23) or the pair (k, k+1) for odd k."""
                if k in (0, KT - 1):
                    t = xt_pool.tile([128, BL], F16, name=f"x_{k}", tag="x",
                                     bufs=2)
                    nc.sync.dma_start(t[:], xt_r[:, k, :])
                    xsolo[k] = t
                else:
                    assert k % 2 == 1
                    t = xt_pool.tile([128, 2, BL], F16, name=f"x_{k}",
                                     tag="x2", bufs=11)
                    nc.sync.dma_start(t[:], xt_r[:, k:k + 2, :])
                    xpair[(k - 1) // 2] = t

            def xt_ap(k, n):
                """[128, NFREE] view of xt k-tile k, b-chunk n."""
                sf = slice(n * NFREE, (n + 1) * NFREE)
                if k in (0, KT - 1):
                    return xsolo[k][:, sf]
                return xpair[(k - 1) // 2][:, (k - 1) % 2, sf]

            def load_w(kind, m, pool=None, tag="w", eng=None):
                """Load the weight block for (kind, m) as KC-k-tile chunks."""
                nk, _ = WKINDS[kind]
                chunks = []
                for c in range(0, nk, KC):
                    nkc = min(KC, nk - c)
                    t = (pool or w_pool).tile([128, nkc, 128], F16,
                                              name=f"w_{kind}_{m}_{c}",
                                              tag=tag)
                    (eng or nc.sync).dma_start(
                        t[:],
                        w_d[kind][m][:, c * 128:(c + nkc) * 128]
                        .rearrange("p (k c) -> p k c", k=nkc))
                    chunks.append(t)
                return chunks

            # Early DMAs in strict first-use order, all <=256KB.  pf weights
            # live in const_pool (per-m tags) so they never contend with the
            # main-phase weight ring.  (Tried issuing the first two loads on
            # the gpsimd queue, which wakes ~1.4us earlier — but its SWDGE
            # software descriptor generation made them land ~2us LATER than
            # sync-queue HWDGE issues.)
            w_pfs = [None] * MT
            w_pfs[0] = load_w("pf", 0, pool=const_pool, tag="wpf0")
            load_x(0)
            w_pfs[1] = load_w("pf", 1, pool=const_pool, tag="wpf1")
            load_x(1)
            load_x(3)
            w_pfs[2] = load_w("pf", 2, pool=const_pool, tag="wpf2")
            w_pfs[3] = load_w("pf", 3, pool=const_pool, tag="wpf3")
            load_x(5)
            load_x(7)
            for m in range(4, MT):
                w_pfs[m] = load_w("pf", m, pool=const_pool, tag=f"wpf{m}")

            bt_t = const_pool.tile([128, 5 * MT], F32, name="bt_t", tag="bt")
            nc.sync.dma_start(bt_t[:], bt_d)

            # m=0 forget-gate weights next (first thing the main phase
            # needs), then the rest of XT, then the host-built Strassen
            # moving-side combos (first used by m=0's i/u multiplies at
            # ~55us).
            w_fl0 = load_w("fl", 0)
            w_fr0 = load_w("fr", 0)
            cb_t = {}

            def load_cb(j):
                t = const_pool.tile([128, KS, NFREE], F16, name=f"cb{j}",
                                    tag=f"cb{j}")
                nc.sync.dma_start(t[:], cb_d[j].rearrange("k p n -> p k n"))
                cb_t[j] = t

            def load_ws(s, m):
                t = w_pool.tile([128, KS, 128], F16,
                                name=f"w_s{s}_{m}", tag="ws", bufs=4)
                nc.sync.dma_start(
                    t[:],
                    w_d[f"s{s}"][m].rearrange("p (k c) -> p k c", k=KS))
                return t

            def load_wso(s, j):
                t = w_pool.tile([128, KS, 128], F16,
                                name=f"w_so{s}_{j}", tag="ws", bufs=4)
                nc.sync.dma_start(
                    t[:],
                    so_d[s][j].rearrange("p (k c) -> p k c", k=KS))
                return t

            # Phase O's first strassen weight blocks go out BEFORE the
            # 7.5MB of cb combos so they aren't bandwidth-starved when the
            # o-strassen phase starts (~45us).  Exactly ring-capacity tiles
            # are preloaded so no issue blocks the sync queue on a gemm WAR.
            w_s0 = {}
            for k in (9, 11, 13, 15):
                load_x(k)
            for k in (17, 19, 21, 23):
                load_x(k)
            w_s0[2] = load_ws(2, 0)
            w_s0[5] = load_ws(5, 0)
            load_cb(1)
            w_s0[1] = load_ws(1, 0)
            load_cb(2)
            for j in (3, 4, 5):
                load_cb(j)

            def s_rhs(s, k):
                """Moving operand for strassen multiply s, k-tile k."""
                kind, arg = S_RHS[s]
                if kind == "cb":
                    return cb_t[arg][:, k, :]
                if arg == 0:
                    return xt_ap(k, 0)          # B11
                return xt_ap(KS + k, 1)         # B22

            def pf_w(m, k):
                return w_pfs[m][0][:, k, :]

            def gemm(kind, m, w_chunks, n_outer=False):
                """Accumulate the gate block, return NT psum tiles.

                n_outer=True finishes all of n=0 before starting n=1 so
                n=0's eviction chain overlaps n=1's matmuls (tail shave
                for the final gate).
                """
                nk, koff = WKINDS[kind]
                ps = [p_pool.tile([128, NFREE], F32,
                                  name=f"ps_{kind}_{m}_{n}", tag="ps")
                      for n in range(NT)]
                if n_outer:
                    for n in range(NT):
                        for k in range(nk):
                            nc.tensor.matmul(
                                ps[n][:], w_chunks[k // KC][:, k % KC, :],
                                xt_ap(koff + k, n),
                                start=(k == 0), stop=(k == nk - 1))
                else:
                    for k in range(nk):
                        w_t = w_chunks[k // KC]
                        for n in range(NT):
                            nc.tensor.matmul(
                                ps[n][:], w_t[:, k % KC, :],
                                xt_ap(koff + k, n),
                                start=(k == 0), stop=(k == nk - 1))
                return ps

            # Phase 1: all pf gemms (only consume XT k-tiles 0..7).  Their
            # SBUF results stay resident until each m's forget gates run.
            # m-pairs with k outermost: each new xt k-tile feeds 4 MMs
            # (~0.86us at full clock), matching the ~0.7us/tile DMA
            # delivery rate during the ramp, so the pf phase never stalls
            # on xt arrivals (m-outer consumed a tile per 0.43us and lost
            # ~3us to stalls + HAM clock re-gating).
            pf_sbs = [None] * MT
            for m0 in range(0, MT, 2):
                pair = (m0, m0 + 1)
                ps_pf = {}
                for m in pair:
                    for n in range(NT):
                        ps_pf[(m, n)] = p_pool.tile(
                            [128, NFREE], F32,
                            name=f"ps_pf_{m}_{n}", tag="ps")
                for k in range(KC):
                    for m in pair:
                        for n in range(NT):
                            nc.tensor.matmul(
                                ps_pf[(m, n)][:], pf_w(m, k), xt_ap(k, n),
                                start=(k == 0), stop=(k == KC - 1))
                for m in pair:
                    pf_sb = []
                    for n in range(NT):
                        t = g_pool.tile([128, NFREE], F16,
                                        name=f"pf_{m}_{n}", tag="pf",
                                        bufs=MT * NT)
                        nc.scalar.copy(t[:], ps_pf[(m, n)][:])
                        pf_sb.append(t)
                    pf_sbs[m] = pf_sb

            # combination term tables: for a strassen M-half pair (A, B),
            # per half and n-chunk: (first two signed Mi, optional last two)
            S_TERMS = ((((1, 4), (-5, 7)), ((3, 5), None)),
                       (((2, 4), None), ((1, -2), (3, 6))))

            def s_combine(mi, half, n):
                pos, extra = S_TERMS[half][n]
                z = e_pool.tile([128, NFREE], F16,
                                name=f"zc_{half}_{n}", tag="zs", bufs=2)
                a, b = pos
                if b < 0:
                    nc.vector.tensor_sub(z[:], mi[a][:], mi[-b][:])
                else:
                    nc.vector.tensor_add(z[:], mi[a][:], mi[b][:])
                if extra is not None:
                    for t_ in extra:
                        if t_ < 0:
                            nc.vector.tensor_sub(z[:], z[:], mi[-t_][:])
                        else:
                            nc.vector.tensor_add(z[:], z[:], mi[t_][:])
                return z

            for m in range(MT):
                pf_sb = pf_sbs[m]
                w_fl = w_fl0 if m == 0 else load_w("fl", m)
                w_fr = w_fr0 if m == 0 else load_w("fr", m)
                # strassen weights as single 12-k-tile chunks in their own
                # ring: fewer/bigger DMAs with deeper prefetch (21 small
                # chunks/m against an 8-slot ring serialized issues and
                # left one group-start LDW waiting ~2.6us on its weights
                # every m-tile)
                # load in gemm-consumption order (2,5,1,3,4,6,7): with a
                # 3-slot ring, loading s1..s7 in numeric order deadlocks
                # (s4's issue blocks the sync FIFO on s1's gemm, which is
                # queued after s5's gemm, whose weights sit behind s4)
                w_s = {}
                for s in (2, 5, 1, 3, 4, 6, 7):
                    if m == 0 and s in w_s0:
                        w_s[s] = [w_s0[s]]
                    else:
                        w_s[s] = [load_ws(s, m)]
                w_so = {}
                if m < 3:
                    for s in (2, 5, 1, 3, 4, 6, 7):
                        w_so[s] = load_wso(s, m)
                plain_o = m in (3, MT - 1)
                if plain_o:
                    w_o = load_w("o", m)
                elif m >= 4:
                    gor = e_pool.tile([128, BL], F16, name=f"gor_{m}",
                                      tag="gor", bufs=1)
                    nc.sync.dma_start(gor[:], gsp_d[m])

                gates = {}
                zf_tiles = {}
                for kind, w_t in (("fl", w_fl), ("fr", w_fr)):
                    ps = gemm(kind, m, w_t)
                    for n in range(NT):
                        # the DVE add evicts the PSUM tile immediately; the
                        # sigmoid is deferred until after the strassen
                        # section so the mi PSUM-evictions sit at the HEAD
                        # of the ACT queue (a WAR-blocked matmul waiting on
                        # an ACT eviction queued behind gate sigmoids cost
                        # ~2.6us per m-tile)
                        z = e_pool.tile([128, NFREE], F16,
                                        name=f"z_{kind}_{m}_{n}",
                                        tag="zf", bufs=4)
                        nc.vector.tensor_add(z[:], ps[n][:], pf_sb[n][:])
                        zf_tiles[(kind, n)] = z

                # i and u via one Strassen level: 7 multiplies of KS=12
                # k-tiles each, partial products evicted to fp16 SBUF and
                # combined on DVE into the 4 pre-activations
                # (zi_n0=M1+M4-M5+M7, zi_n1=M3+M5, zu_n0=M2+M4,
                #  zu_n1=M1-M2+M3+M6).
                mi = {}
                for s in (2, 5, 1, 3, 4, 6, 7):
                    ps_s = p_pool.tile([128, NFREE], F32,
                                       name=f"ps_s{s}_{m}", tag="ps")
                    wch = w_s[s][0]
                    for k in range(KS):
                        nc.tensor.matmul(
                            ps_s[:], wch[:, k, :], s_rhs(s, k),
                            start=(k == 0), stop=(k == KS - 1))
                    t = g_pool.tile([128, NFREE], F16, name=f"mi_{s}_{m}",
                                    tag=f"mi{s}", bufs=1)
                    nc.scalar.copy(t[:], ps_s[:])
                    mi[s] = t

                # deferred forget-gate activations (ACT, after mi copies)
                for kind in ("fl", "fr"):
                    bi = BIAS_IDX[kind]
                    for n in range(NT):
                        gt = g_pool.tile([128, NFREE], F16,
                                         name=f"g_{kind}_{m}_{n}",
                                         tag=f"g{kind}", bufs=2)
                        nc.scalar.activation(
                            gt[:], zf_tiles[(kind, n)][:], SIG,
                            bias=bt_t[:, bi * MT + m: bi * MT + m + 1])
                        gates[(kind, n)] = gt

                for gate, act, bi, terms in (
                        ("i", SIG, 0, (((1, 4), (-5, 7)), ((3, 5), None))),
                        ("u", TANH, 3, (((2, 4), None), ((1, -2), (3, 6))))):
                    for n in range(NT):
                        pos, extra = terms[n]
                        z = e_pool.tile([128, NFREE], F16,
                                        name=f"z_{gate}_{m}_{n}", tag="zs",
                                        bufs=2)
                        a, b = pos
                        if b < 0:
                            nc.vector.tensor_sub(z[:], mi[a][:], mi[-b][:])
                        else:
                            nc.vector.tensor_add(z[:], mi[a][:], mi[b][:])
                        if extra is not None:
                            c1_, c2_ = extra
                            if c1_ < 0:
                                nc.vector.tensor_sub(z[:], z[:], mi[-c1_][:])
                            else:
                                nc.vector.tensor_add(z[:], z[:], mi[c1_][:])
                            if c2_ < 0:
                                nc.vector.tensor_sub(z[:], z[:], mi[-c2_][:])
                            else:
                                nc.vector.tensor_add(z[:], z[:], mi[c2_][:])
                        gt = g_pool.tile([128, NFREE], F16,
                                         name=f"g_{gate}_{m}_{n}",
                                         tag=f"g{gate}", bufs=2)
                        nc.scalar.activation(
                            gt[:], z[:], act,
                            bias=bt_t[:, bi * MT + m: bi * MT + m + 1])
                        gates[(gate, n)] = gt

                go_sb = {}
                if m < 3:
                    # o-gate strassen for the pair (m, m+4), interleaved
                    # into the m-loop so its weight traffic spreads across
                    # the iteration instead of colliding with the early
                    # cb/xt DMAs (a standalone phase-O stalled 22us).
                    mi_o = {}
                    for s_ in (2, 5, 1, 3, 4, 6, 7):
                        wso = w_so[s_]
                        ps_s = p_pool.tile([128, NFREE], F32,
                                           name=f"ps_so{s_}_{m}", tag="ps")
                        for k in range(KS):
                            nc.tensor.matmul(
                                ps_s[:], wso[:, k, :], s_rhs(s_, k),
                                start=(k == 0), stop=(k == KS - 1))
                        t = g_pool.tile([128, NFREE], F16,
                                        name=f"mo_{s_}_{m}",
                                        tag=f"mi{s_}", bufs=1)
                        nc.scalar.copy(t[:], ps_s[:])
                        mi_o[s_] = t
                    for half, mm in ((0, m), (1, m + 4)):
                        for n in range(NT):
                            z = s_combine(mi_o, half, n)
                            go = e_pool.tile([128, NFREE], F16,
                                             name=f"go_{mm}_{n}", tag="go")
                            nc.scalar.activation(
                                go[:], z[:], SIG,
                                bias=bt_t[:, 4 * MT + mm: 4 * MT + mm + 1])
                            if half == 0:
                                go_sb[n] = go
                            else:
                                nc.sync.dma_start(
                                    gsp_d[mm][:, n * NFREE:(n + 1) * NFREE],
                                    go[:])

                # c-chain: independent of o, overlaps o's matmuls.  cl/cr
                # arrive fp16 and the fl*cl / fr*cr products overwrite them
                # in place (saves two fp32 scratch tags of SBUF).
                th_tiles = {}
                for n in range(NT):
                    sp = slice(m * 128, (m + 1) * 128)
                    sf = slice(n * NFREE, (n + 1) * NFREE)
                    cl_t = e_pool.tile([128, NFREE], F16,
                                       name=f"cl_{m}_{n}", tag="cl")
                    nc.sync.dma_start(cl_t[:], clt_d[sp, sf])
                    cr_t = e_pool.tile([128, NFREE], F16,
                                       name=f"cr_{m}_{n}", tag="cr")
                    nc.sync.dma_start(cr_t[:], crt_d[sp, sf])

                    iu = e_pool.tile([128, NFREE], F32,
                                     name=f"iu_{m}_{n}", tag="iu")
                    nc.vector.tensor_mul(iu[:], gates[("i", n)][:],
                                         gates[("u", n)][:])
                    nc.vector.tensor_mul(cl_t[:], gates[("fl", n)][:],
                                         cl_t[:])
                    nc.vector.tensor_mul(cr_t[:], gates[("fr", n)][:],
                                         cr_t[:])
                    # c accumulates in-place in iu
                    nc.vector.tensor_add(iu[:], iu[:], cl_t[:])
                    nc.vector.tensor_add(iu[:], iu[:], cr_t[:])
                    nc.sync.dma_start(ct_d[sp, sf], iu[:])

                    th = e_pool.tile([128, NFREE], F16,
                                     name=f"th_{m}_{n}", tag="th")
                    nc.scalar.activation(th[:], iu[:], TANH)
                    th_tiles[n] = th

                if not plain_o:
                    # o came from the in-loop strassen pair (m<3: SBUF;
                    # m>=4: DRAM spill reload)
                    for n in range(NT):
                        sp = slice(m * 128, (m + 1) * 128)
                        sf = slice(n * NFREE, (n + 1) * NFREE)
                        h_t = e_pool.tile([128, NFREE], F32,
                                          name=f"h_{m}_{n}", tag="h")
                        o_ap = go_sb[n][:] if m < 3 else gor[:, sf]
                        nc.vector.tensor_mul(h_t[:], o_ap, th_tiles[n][:])
                        nc.sync.dma_start(ht_d[sp, sf], h_t[:])
                    continue

                ps_o = gemm("o", m, w_o, n_outer=True)
                for n in range(NT):
                    sp = slice(m * 128, (m + 1) * 128)
                    sf = slice(n * NFREE, (n + 1) * NFREE)
                    # chunk the very last eviction's ACT/DVE so they
                    # pipeline on the kernel tail; keep a single DMA (a
                    # second dma_start costs ~0.6us of serialized sync-queue
                    # issue, more than the larger transfer)
                    last = (m == MT - 1 and n == NT - 1)
                    nch = 2 if last else 1
                    cw = NFREE // nch
                    go = e_pool.tile([128, NFREE], F16,
                                     name=f"g_o_{m}_{n}", tag="go")
                    h_t = e_pool.tile([128, NFREE], F32,
                                      name=f"h_{m}_{n}", tag="h")
                    for ci in range(nch):
                        cs = slice(ci * cw, (ci + 1) * cw)
                        nc.scalar.activation(
                            go[:, cs], ps_o[n][:, cs], SIG,
                            bias=bt_t[:, 4 * MT + m: 4 * MT + m + 1])
                        nc.vector.tensor_mul(h_t[:, cs], go[:, cs],
                                             th_tiles[n][:, cs])
                    nc.sync.dma_start(ht_d[sp, sf], h_t[:])

    nc.compile()
    return nc


def _get_program():
    if "nc" not in _CACHE:
        _CACHE["nc"] = _build_program()
    return _CACHE["nc"]


def _tile_weight(V, nk, mt=MT):
    """[nk*128, mt*128] -> [mt, 128, nk*128] with [m][kp, k*128+mc] = V[k*128+kp, m*128+mc]."""
    return np.ascontiguousarray(
        V.reshape(nk, 128, mt, 128)
         .transpose(2, 1, 0, 3)
         .reshape(mt, 128, nk * 128)
         .astype(np.float16))


def kernel(hl, cl, hr, cr, p,
           Wd, Wdl, Wdr, bd,
           Wf, Wfll, Wflr, Wfrl, Wfrr, bfl, bfr,
           Wo, Wol, Wor, bo,
           Wi, Wil, Wir, bi):
    global LAST_RESULTS
    f32 = np.float32
    hl, cl, hr, cr, p = (np.asarray(a, dtype=f32) for a in (hl, cl, hr, cr, p))
    ws = {k: np.asarray(v, dtype=f32) for k, v in dict(
        Wd=Wd, Wdl=Wdl, Wdr=Wdr, Wf=Wf, Wfll=Wfll, Wflr=Wflr, Wfrl=Wfrl,
        Wfrr=Wfrr, Wo=Wo, Wol=Wol, Wor=Wor, Wi=Wi, Wil=Wil, Wir=Wir).items()}

    # Wf{gate l/r}{child l/r}: f_left mixes hl via Wfll and hr via Wflr;
    # f_right mixes hl via Wfrl and hr via Wfrr.
    # i|u joint weight block [K=3072, M=2048] split into Strassen quadrants
    # (A11=[K1,i], A12=[K1,u], A21=[K2,i], A22=[K2,u]; K1 = first 1536 rows)
    # and combined on the host in fp32 before the fp16 cast.
    Wi_blk = np.concatenate([ws["Wd"].T, ws["Wdl"].T, ws["Wdr"].T], 0)
    Wu_blk = np.concatenate([ws["Wi"].T, ws["Wil"].T, ws["Wir"].T], 0)
    K1 = KS * 128
    A11, A21 = Wi_blk[:K1], Wi_blk[K1:]
    A12, A22 = Wu_blk[:K1], Wu_blk[K1:]
    s_w = {1: A11 + A22, 2: A12 + A22, 3: A11, 4: A22,
           5: A11 + A21, 6: A12 - A11, 7: A21 - A22}
    wt = {
        "o": _tile_weight(np.concatenate(
            [ws["Wo"].T, ws["Wol"].T, ws["Wor"].T], 0), KT),
        "pf": _tile_weight(np.ascontiguousarray(ws["Wf"].T), 8),
        "fl": _tile_weight(np.concatenate(
            [ws["Wfll"].T, ws["Wflr"].T], 0), 16),
        "fr": _tile_weight(np.concatenate(
            [ws["Wfrl"].T, ws["Wfrr"].T], 0), 16),
    }
    for s, Vw in s_w.items():
        wt[f"s{s}"] = _tile_weight(np.ascontiguousarray(Vw), KS)
    # o-gate strassen quadrants: M-half = 512 (4 m-tiles)
    Wo_blk = np.concatenate([ws["Wo"].T, ws["Wol"].T, ws["Wor"].T], 0)
    O11, O12 = Wo_blk[:K1, :512], Wo_blk[:K1, 512:]
    O21, O22 = Wo_blk[K1:, :512], Wo_blk[K1:, 512:]
    so_w = {1: O11 + O22, 2: O12 + O22, 3: O11, 4: O22,
            5: O11 + O21, 6: O12 - O11, 7: O21 - O22}
    so_t = {s: _tile_weight(np.ascontiguousarray(Vw), KS, mt=4)
            for s, Vw in so_w.items()}

    Bt = np.empty((128, 5 * MT), dtype=f32)
    for name, b_ in (("i", bd), ("fl", bfl), ("fr", bfr), ("u", bi), ("o", bo)):
        gi = BIAS_IDX[name]
        Bt[:, gi * MT:(gi + 1) * MT] = np.asarray(b_, dtype=f32).reshape(MT, 128).T

    X = np.concatenate([p, hl, hr], axis=1)    # [B, 3D]

    f16 = np.float16
    in_maps = []
    for r in range(NCORES):
        rows = slice(r * BL, (r + 1) * BL)
        XT = X[rows].T.astype(f16)                    # [3D, BL] fp16
        XTf = XT.astype(f32)
        N1 = NFREE
        B11, B12 = XTf[:K1, :N1], XTf[:K1, N1:]
        B21, B22 = XTf[K1:, :N1], XTf[K1:, N1:]
        cbs = {1: B11 + B22, 2: B12 - B22, 3: B21 - B11,
               4: B11 + B12, 5: B21 + B22}
        im = {
            "xt": np.ascontiguousarray(XT),
            "clt": np.ascontiguousarray(cl[rows].T.astype(f16)),
            "crt": np.ascontiguousarray(cr[rows].T.astype(f16)),
            "bt": Bt,
        }
        for j, cb in cbs.items():
            im[f"cb{j}"] = np.ascontiguousarray(
                cb.astype(f16).reshape(KS, 128, NFREE))
        for kind, arr in wt.items():
            im[f"w_{kind}"] = arr
        for s, arr in so_t.items():
            im[f"so{s}"] = arr
        in_maps.append(im)

    nc = _get_program()
    res = bass_utils.run_bass_kernel_spmd(nc, in_maps,
                                          core_ids=list(range(NCORES)))
    LAST_RESULTS = res

    h = np.empty((B, H), dtype=f32)
    c = np.empty((B, H), dtype=f32)
    for r in range(NCORES):
        rows = slice(r * BL, (r + 1) * BL)
        h[rows] = res.results[r]["ht"].T
        c[rows] = res.results[r]["ct"].T
    return (h, c)


# revision 28
# speedup vs baseline: 1.1887x; 1.0031x over previous
"""BinaryLSTM (binary tree-LSTM cell) Trainium2 kernel.

Full-input contract: kernel(**inputs) takes the complete unsharded tensors and
returns (h, c), each [8192, 1024] float32, matching the reference.

Strategy
--------
Data-parallel over the batch dim: core r handles rows r*1024:(r+1)*1024.
The 14 weight matrices are fused on the host into per-gate blocks over the
concatenated input X = [p | hl | hr] ([B, 3072]).  Gate pre-activations are
computed as z[h, b] so the contraction dim sits on SBUF partitions:

  z_g[h, b] = sum_k Vg[k, h] * XT[k, b]   (lhsT = Vg tile, rhs = XT tile)

PSUM tiles are [h_part=128, b_free=512] and the per-gate bias (varying along
h) is a per-partition [128,1] bias fused into the ACT sigmoid/tanh.

Matmuls run in float16 (full PE rate, ~216ns per 128x128x512 MM vs ~227ns
for float32r; quantization error ~8e-4 on h, measured in simulation).  PSUM
accumulation and everything downstream is fp32.

The kernel is PE-bound: steady-state MM pitch is 215.8ns = 512+6 cycles at
2.4GHz, the hardware floor for N=512 fp32-PSUM matmuls (N=1024 would cross
a PSUM bank, which a single matmul cannot).  Ramp mitigations (the engine
queues only start executing user instructions at ~6-7.4us after a fixed
framework preamble of barriers + register loads, and the first DMA lands
~11us in: 7.4us sync-queue preamble + 0.65us/dma_start issue + ~2us fixed
DMA latency + transfer):
  * NWARM dummy warm-up matmuls on a gpsimd-zeroed SBUF tile start at
    ~6.5us with no DMA dependency and bridge until the first real gemm's
    data lands (~14us).  This keeps the HAM clock-gate ramp (PE starts at
    ~4/8 clock, reaches 8/8 after ~4us of busy time) overlapped with the
    DMA pipeline fill, and avoids PE-idle gaps >~2us, which re-throttle
    the clock to 4/8 for 3.4us.
  * Early DMAs are 256-512KB, issued in strict first-use order (pf-weights
    m=0, xt[0], pf-weights m=1, xt[1..2], ...).  Concurrent transfers
    share the ~358GB/s HBM port fairly, so a big out-of-order transfer
    delays everything behind it (measured: a 2MB block issued 3rd starved
    the pf gemms for 8.5us and re-triggered the clock gate).
  * The pf phase runs m-PAIRS with k outermost, so each newly-arriving xt
    k-tile feeds 4 matmuls (~0.86us) — matching the ~0.7us/tile DMA
    delivery rate — instead of 2 (m-outer lost ~3us to arrival stalls).

The shared forget p-projection (pf = p @ Wf.T) is computed once per tile
(K=1024 accumulation), copied PSUM->SBUF (fp16), and added to the two
forget-gate child projections on DVE.  The i and u gates are computed
jointly via one Strassen level on their [3072, 2048] weight block (M split
at the i/u gate boundary); the o gate gets the same treatment with M split
at 512, pairing output tiles (m, m+4) for m=0..2 — m=3 and m=7 keep plain
o gemms so the kernel tail keeps its n_outer eviction overlap.  Both reuse
the SAME host-built moving-side combos (cb1..cb5); o-gate values for m+4
spill to DRAM scratch (fp16) and reload ~180us later.  Partial products
evict to fp16 SBUF via ACT and recombine on DVE; combine order matters:
the o-gate copies must come AFTER the i/u combines (shared mi rings,
bufs=1) and every weight ring must load in gemm-consumption order or the
scheduler deadlocks.  Total MACs: 12.3*B*D*H vs the 14*B*D*H dense
minimum (~48 matmuls saved per core).  Intermediates that tolerate fp16
(pre-activations, gate values, cl/cr, tanh(c)) are fp16 to fit the
Strassen combo/partial tiles in SBUF; measured rel err 4.6e-3 vs the
2e-2 gate.  The final o-tile eviction is chunked so ACT/DVE pipeline on
the tail.
"""

import os
import sys

for _p in ("/opt/trn_rl_repo", "/root/.axon_site/_ro/trn_rl_repo"):
    if os.path.isdir(_p) and _p not in sys.path:
        sys.path.append(_p)

import numpy as np

import concourse.bass as bass
import concourse.tile as tile
import concourse.mybir as mybir
from concourse import bacc
from concourse import bass_utils

B, D, H = 8192, 1024, 1024
NCORES = 8
BL = B // NCORES            # 1024 batch rows per core
K3 = 3 * D                  # 3072 contraction (p | hl | hr)
KT = K3 // 128              # 24 k-tiles
MT = H // 128               # 8 h-tiles (PSUM partition dim)
NFREE = 512                 # moving free dim per matmul (one PSUM bank, fp32)
NT = BL // NFREE            # 2 b-tiles
KC = 8                      # weight-chunk k-tiles per SBUF weight tile
NWARM = 18                  # clock-warm-up dummy matmuls

F32 = mybir.dt.float32
F16 = mybir.dt.float16

_CACHE = {}

# Results of the most recent hardware run (for test harness introspection).
LAST_RESULTS = None

# weight blocks: (name, #k-tiles, xt k-tile offset)
# i and u are computed jointly via one Strassen level on the [K=3072,
# M=2048] i|u weight block (K split at 1536, M split at 1024 = the i/u
# gate boundary, N split at 512 = the existing n=0/1 split), so each
# 128-row h-block m gets z_i and z_u from 7 multiplies of 12 k-tiles
# (84 MMs) instead of 2 gates x 24 k x 2 n (96 MMs).
KS = 12                    # strassen multiply contraction k-tiles (K/2)
WKINDS = {
    "o": (KT, 0),      # output gate, full K
    "pf": (D // 128, 0),              # shared forget p-projection (p rows)
    "fl": (2 * D // 128, D // 128),   # f_left child projections (hl|hr rows)
    "fr": (2 * D // 128, D // 128),   # f_right child projections
}
for _s in range(1, 8):
    WKINDS[f"s{_s}"] = (KS, 0)
# bias column index per activated gate
BIAS_IDX = {"i": 0, "fl": 1, "fr": 2, "u": 3, "o": 4}
# strassen multiply -> moving operand: ("cb", j) = host combo j, ("x", 0) =
# B11 = xt k 0..11 cols n=0, ("x", 1) = B22 = xt k 12..23 cols n=1
S_RHS = {
    1: ("cb", 1),   # M1 = (A11+A22)^T (B11+B22)
    2: ("x", 0),    # M2 = (A12+A22)^T B11
    3: ("cb", 2),   # M3 = A11^T (B12-B22)
    4: ("cb", 3),   # M4 = A22^T (B21-B11)
    5: ("x", 1),    # M5 = (A11+A21)^T B22
    6: ("cb", 4),   # M6 = (A12-A11)^T (B11+B12)
    7: ("cb", 5),   # M7 = (A21-A22)^T (B21+B22)
}


def _build_program():
    nc = bacc.Bacc("TRN2", target_bir_lowering=False, debug=False,
                   num_devices=NCORES)

    xt_d = nc.dram_tensor("xt", [K3, BL], F16, kind="ExternalInput").ap()
    w_d = {}
    for kind, (nk, _) in WKINDS.items():
        w_d[kind] = nc.dram_tensor(f"w_{kind}", [MT, 128, nk * 128], F16,
                                   kind="ExternalInput").ap()
    cb_d = {j: nc.dram_tensor(f"cb{j}", [KS, 128, NFREE], F16,
                              kind="ExternalInput").ap()
            for j in range(1, 6)}
    # o-gate strassen weights: M-half is 512 (4 m-tiles); pair groups
    # (j, j+4) for j=0..2 — m=3 and m=7 keep plain o gemms so the tail
    # keeps its n_outer overlap structure.
    so_d = {s: nc.dram_tensor(f"so{s}", [4, 128, KS * 128], F16,
                              kind="ExternalInput").ap()
            for s in range(1, 8)}
    gsp_d = nc.dram_tensor("gsp", [MT, 128, BL], F16, kind="Internal").ap()
    clt_d = nc.dram_tensor("clt", [H, BL], F16, kind="ExternalInput").ap()
    crt_d = nc.dram_tensor("crt", [H, BL], F16, kind="ExternalInput").ap()
    bt_d = nc.dram_tensor("bt", [128, 5 * MT], F32, kind="ExternalInput").ap()
    ht_d = nc.dram_tensor("ht", [H, BL], F32, kind="ExternalOutput").ap()
    ct_d = nc.dram_tensor("ct", [H, BL], F32, kind="ExternalOutput").ap()

    SIG = mybir.ActivationFunctionType.Sigmoid
    TANH = mybir.ActivationFunctionType.Tanh

    with tile.TileContext(nc) as tc:
        with tc.tile_pool(name="const", bufs=1) as const_pool, \
             tc.tile_pool(name="xtp", bufs=KT) as xt_pool, \
             tc.tile_pool(name="wp", bufs=6) as w_pool, \
             tc.tile_pool(name="gp", bufs=1) as g_pool, \
             tc.tile_pool(name="ep", bufs=2) as e_pool, \
             tc.tile_pool(name="pp", bufs=8, space="PSUM") as p_pool:

            # ---- PE clock warm-up: dummy matmuls with no DMA dependency ----
            # gpsimd memset: the gpsimd queue clears its framework preamble
            # first (~5.9us), so the warm matmuls start ~1.5us earlier than
            # with a DVE memset.
            warm = const_pool.tile([128, NFREE], F16, name="warm", tag="warm")
            nc.gpsimd.memset(warm[:], 0.0)
            ps_warm = p_pool.tile([128, NFREE], F32, name="ps_warm", tag="ps")
            for wi in range(NWARM):
                nc.tensor.matmul(ps_warm[:], warm[:, 0:128], warm[:],
                                 start=(wi == 0), stop=(wi == NWARM - 1))

            xt_r = xt_d.rearrange("(k p) b -> p k b", p=128)
            # xt k-tiles load as [k0 solo, 11 pairs, k23 solo] so the pf
            # phase's first dependency is small (256KB) but later tiles
            # arrive via half as many dma_start issues (each costs ~0.65us
            # of sync-queue time plus ~2us fixed DMA latency; per-tile
            # loads left k=3..7 arriving ~1us after the pf gemms wanted
            # them).
            xpair = [None] * 11
            xsolo = {}

            def load_x(k):
                """Load xt tile k (k=0,23) or the pair (k, k+1) for odd k."""
                if k in (0, KT - 1):
                    t = xt_pool.tile([128, BL], F16, name=f"x_{k}", tag="x",
                                     bufs=2)
                    nc.sync.dma_start(t[:], xt_r[:, k, :])
                    xsolo[k] = t
                else:
                    assert k % 2 == 1
                    t = xt_pool.tile([128, 2, BL], F16, name=f"x_{k}",
                                     tag="x2", bufs=11)
                    if k < 8:
                        # pf-phase-critical pairs load per-tile so subtile
                        # deps release k's matmuls when its half lands
                        # (pair-granular arrival cost 1.5-3us of pf stalls)
                        nc.sync.dma_start(t[:, 0, :], xt_r[:, k, :])
                        nc.sync.dma_start(t[:, 1, :], xt_r[:, k + 1, :])
                    else:
                        nc.sync.dma_start(t[:], xt_r[:, k:k + 2, :])
                    xpair[(k - 1) // 2] = t

            def xt_ap(k, n):
                """[128, NFREE] view of xt k-tile k, b-chunk n."""
                sf = slice(n * NFREE, (n + 1) * NFREE)
                if k in (0, KT - 1):
                    return xsolo[k][:, sf]
                return xpair[(k - 1) // 2][:, (k - 1) % 2, sf]

            def load_w(kind, m, pool=None, tag="w", eng=None):
                """Load the weight block for (kind, m) as KC-k-tile chunks."""
                nk, _ = WKINDS[kind]
                chunks = []
                for c in range(0, nk, KC):
                    nkc = min(KC, nk - c)
                    t = (pool or w_pool).tile([128, nkc, 128], F16,
                                              name=f"w_{kind}_{m}_{c}",
                                              tag=tag)
                    (eng or nc.sync).dma_start(
                        t[:],
                        w_d[kind][m][:, c * 128:(c + nkc) * 128]
                        .rearrange("p (k c) -> p k c", k=nkc))
                    chunks.append(t)
                return chunks

            # Early DMAs in strict first-use order, all <=256KB.  pf weights
            # live in const_pool (per-m tags) so they never contend with the
            # main-phase weight ring.  (Tried issuing the first two loads on
            # the gpsimd queue, which wakes ~1.4us earlier — but its SWDGE
            # software descriptor generation made them land ~2us LATER than
            # sync-queue HWDGE issues.)
            w_pfs = [None] * MT
            w_pfs[0] = load_w("pf", 0, pool=const_pool, tag="wpf0")
            load_x(0)
            w_pfs[1] = load_w("pf", 1, pool=const_pool, tag="wpf1")
            load_x(1)
            load_x(3)
            w_pfs[2] = load_w("pf", 2, pool=const_pool, tag="wpf2")
            w_pfs[3] = load_w("pf", 3, pool=const_pool, tag="wpf3")
            load_x(5)
            load_x(7)
            for m in range(4, MT):
                w_pfs[m] = load_w("pf", m, pool=const_pool, tag=f"wpf{m}")

            bt_t = const_pool.tile([128, 5 * MT], F32, name="bt_t", tag="bt")
            nc.sync.dma_start(bt_t[:], bt_d)

            # m=0 forget-gate weights next (first thing the main phase
            # needs), then the rest of XT, then the host-built Strassen
            # moving-side combos (first used by m=0's i/u multiplies at
            # ~55us).
            w_fl0 = load_w("fl", 0)
            w_fr0 = load_w("fr", 0)
            cb_t = {}

            def load_cb(j):
                t = const_pool.tile([128, KS, NFREE], F16, name=f"cb{j}",
                                    tag=f"cb{j}")
                # two half-loads: subtile deps let the consuming gemm's
                # k<6 matmuls start ~2us before the full 1.5MB lands
                h = KS // 2
                nc.sync.dma_start(t[:, :h, :],
                                  cb_d[j][:h].rearrange("k p n -> p k n"))
                nc.sync.dma_start(t[:, h:, :],
                                  cb_d[j][h:].rearrange("k p n -> p k n"))
                cb_t[j] = t

            def load_ws(s, m):
                t = w_pool.tile([128, KS, 128], F16,
                                name=f"w_s{s}_{m}", tag="ws", bufs=4)
                nc.sync.dma_start(
                    t[:],
                    w_d[f"s{s}"][m].rearrange("p (k c) -> p k c", k=KS))
                return t

            def load_wso(s, j, split=False):
                t = w_pool.tile([128, KS, 128], F16,
                                name=f"w_so{s}_{j}", tag="ws", bufs=4)
                if split:
                    h = KS // 2
                    nc.sync.dma_start(
                        t[:, :h, :],
                        so_d[s][j][:, :h * 128]
                        .rearrange("p (k c) -> p k c", k=h))
                    nc.sync.dma_start(
                        t[:, h:, :],
                        so_d[s][j][:, h * 128:]
                        .rearrange("p (k c) -> p k c", k=h))
                else:
                    nc.sync.dma_start(
                        t[:],
                        so_d[s][j].rearrange("p (k c) -> p k c", k=KS))
                return t

            # Phase O's first strassen weight blocks go out BEFORE the
            # 7.5MB of cb combos so they aren't bandwidth-starved when the
            # o-strassen phase starts (~45us).  Exactly ring-capacity tiles
            # are preloaded so no issue blocks the sync queue on a gemm WAR.
            w_s0 = {}
            for k in (9, 11, 13, 15):
                load_x(k)
            for k in (17, 19, 21, 23):
                load_x(k)
            w_s0[2] = load_ws(2, 0)
            w_s0[5] = load_ws(5, 0)
            load_cb(1)
            w_s0[1] = load_ws(1, 0)
            load_cb(2)
            for j in (3, 4, 5):
                load_cb(j)

            def s_rhs(s, k):
                """Moving operand for strassen multiply s, k-tile k."""
                kind, arg = S_RHS[s]
                if kind == "cb":
                    return cb_t[arg][:, k, :]
                if arg == 0:
                    return xt_ap(k, 0)          # B11
                return xt_ap(KS + k, 1)         # B22

            def pf_w(m, k):
                return w_pfs[m][0][:, k, :]

            def gemm(kind, m, w_chunks, n_outer=False):
                """Accumulate the gate block, return NT psum tiles.

                n_outer=True finishes all of n=0 before starting n=1 so
                n=0's eviction chain overlaps n=1's matmuls (tail shave
                for the final gate).
                """
                nk, koff = WKINDS[kind]
                ps = [p_pool.tile([128, NFREE], F32,
                                  name=f"ps_{kind}_{m}_{n}", tag="ps")
                      for n in range(NT)]
                if n_outer:
                    for n in range(NT):
                        for k in range(nk):
                            nc.tensor.matmul(
                                ps[n][:], w_chunks[k // KC][:, k % KC, :],
                                xt_ap(koff + k, n),
                                start=(k == 0), stop=(k == nk - 1))
                else:
                    for k in range(nk):
                        w_t = w_chunks[k // KC]
                        for n in range(NT):
                            nc.tensor.matmul(
                                ps[n][:], w_t[:, k % KC, :],
                                xt_ap(koff + k, n),
                                start=(k == 0), stop=(k == nk - 1))
                return ps

            # Phase 1: all pf gemms (only consume XT k-tiles 0..7).  Their
            # SBUF results stay resident until each m's forget gates run.
            # m-pairs with k outermost: each new xt k-tile feeds 4 MMs
            # (~0.86us at full clock), matching the ~0.7us/tile DMA
            # delivery rate during the ramp, so the pf phase never stalls
            # on xt arrivals (m-outer consumed a tile per 0.43us and lost
            # ~3us to stalls + HAM clock re-gating).
            pf_sbs = [None] * MT
            for m0 in range(0, MT, 2):
                pair = (m0, m0 + 1)
                ps_pf = {}
                for m in pair:
                    for n in range(NT):
                        ps_pf[(m, n)] = p_pool.tile(
                            [128, NFREE], F32,
                            name=f"ps_pf_{m}_{n}", tag="ps")
                for k in range(KC):
                    for m in pair:
                        for n in range(NT):
                            nc.tensor.matmul(
                                ps_pf[(m, n)][:], pf_w(m, k), xt_ap(k, n),
                                start=(k == 0), stop=(k == KC - 1))
                for m in pair:
                    pf_sb = []
                    for n in range(NT):
                        t = g_pool.tile([128, NFREE], F16,
                                        name=f"pf_{m}_{n}", tag="pf",
                                        bufs=MT * NT)
                        nc.scalar.copy(t[:], ps_pf[(m, n)][:])
                        pf_sb.append(t)
                    pf_sbs[m] = pf_sb

            # combination term tables: for a strassen M-half pair (A, B),
            # per half and n-chunk: (first two signed Mi, optional last two)
            S_TERMS = ((((1, 4), (-5, 7)), ((3, 5), None)),
                       (((2, 4), None), ((1, -2), (3, 6))))

            def s_combine(mi, half, n):
                pos, extra = S_TERMS[half][n]
                z = e_pool.tile([128, NFREE], F16,
                                name=f"zc_{half}_{n}", tag="zs", bufs=2)
                a, b = pos
                if b < 0:
                    nc.vector.tensor_sub(z[:], mi[a][:], mi[-b][:])
                else:
                    nc.vector.tensor_add(z[:], mi[a][:], mi[b][:])
                if extra is not None:
                    for t_ in extra:
                        if t_ < 0:
                            nc.vector.tensor_sub(z[:], z[:], mi[-t_][:])
                        else:
                            nc.vector.tensor_add(z[:], z[:], mi[t_][:])
                return z

            for m in range(MT):
                pf_sb = pf_sbs[m]
                w_fl = w_fl0 if m == 0 else load_w("fl", m)
                w_fr = w_fr0 if m == 0 else load_w("fr", m)
                # strassen weights as single 12-k-tile chunks in their own
                # ring: fewer/bigger DMAs with deeper prefetch (21 small
                # chunks/m against an 8-slot ring serialized issues and
                # left one group-start LDW waiting ~2.6us on its weights
                # every m-tile)
                # load in gemm-consumption order (2,5,1,3,4,6,7): with a
                # 3-slot ring, loading s1..s7 in numeric order deadlocks
                # (s4's issue blocks the sync FIFO on s1's gemm, which is
                # queued after s5's gemm, whose weights sit behind s4)
                w_s = {}
                for s in (2, 5, 1, 3, 4, 6, 7):
                    if m == 0 and s in w_s0:
                        w_s[s] = [w_s0[s]]
                    else:
                        w_s[s] = [load_ws(s, m)]
                w_so = {}
                if m < 3:
                    for s in (2, 5, 1, 3, 4, 6, 7):
                        w_so[s] = load_wso(s, m, split=(m == 0))
                plain_o = m in (3, MT - 1)
                if plain_o:
                    w_o = load_w("o", m)
                elif m >= 4:
                    gor = e_pool.tile([128, BL], F16, name=f"gor_{m}",
                                      tag="gor", bufs=1)
                    nc.sync.dma_start(gor[:], gsp_d[m])

                gates = {}
                zf_tiles = {}
                for kind, w_t in (("fl", w_fl), ("fr", w_fr)):
                    ps = gemm(kind, m, w_t)
                    for n in range(NT):
                        # the DVE add evicts the PSUM tile immediately; the
                        # sigmoid is deferred until after the strassen
                        # section so the mi PSUM-evictions sit at the HEAD
                        # of the ACT queue (a WAR-blocked matmul waiting on
                        # an ACT eviction queued behind gate sigmoids cost
                        # ~2.6us per m-tile)
                        z = e_pool.tile([128, NFREE], F16,
                                        name=f"z_{kind}_{m}_{n}",
                                        tag="zf", bufs=4)
                        nc.vector.tensor_add(z[:], ps[n][:], pf_sb[n][:])
                        zf_tiles[(kind, n)] = z

                # i and u via one Strassen level: 7 multiplies of KS=12
                # k-tiles each, partial products evicted to fp16 SBUF and
                # combined on DVE into the 4 pre-activations
                # (zi_n0=M1+M4-M5+M7, zi_n1=M3+M5, zu_n0=M2+M4,
                #  zu_n1=M1-M2+M3+M6).
                mi = {}
                for s in (2, 5, 1, 3, 4, 6, 7):
                    ps_s = p_pool.tile([128, NFREE], F32,
                                       name=f"ps_s{s}_{m}", tag="ps")
                    wch = w_s[s][0]
                    for k in range(KS):
                        nc.tensor.matmul(
                            ps_s[:], wch[:, k, :], s_rhs(s, k),
                            start=(k == 0), stop=(k == KS - 1))
                    t = g_pool.tile([128, NFREE], F16, name=f"mi_{s}_{m}",
                                    tag=f"mi{s}", bufs=1)
                    nc.scalar.copy(t[:], ps_s[:])
                    mi[s] = t

                # deferred forget-gate activations (ACT, after mi copies)
                for kind in ("fl", "fr"):
                    bi = BIAS_IDX[kind]
                    for n in range(NT):
                        gt = g_pool.tile([128, NFREE], F16,
                                         name=f"g_{kind}_{m}_{n}",
                                         tag=f"g{kind}", bufs=2)
                        nc.scalar.activation(
                            gt[:], zf_tiles[(kind, n)][:], SIG,
                            bias=bt_t[:, bi * MT + m: bi * MT + m + 1])
                        gates[(kind, n)] = gt

                for gate, act, bi, terms in (
                        ("i", SIG, 0, (((1, 4), (-5, 7)), ((3, 5), None))),
                        ("u", TANH, 3, (((2, 4), None), ((1, -2), (3, 6))))):
                    for n in range(NT):
                        pos, extra = terms[n]
                        z = e_pool.tile([128, NFREE], F16,
                                        name=f"z_{gate}_{m}_{n}", tag="zs",
                                        bufs=2)
                        a, b = pos
                        if b < 0:
                            nc.vector.tensor_sub(z[:], mi[a][:], mi[-b][:])
                        else:
                            nc.vector.tensor_add(z[:], mi[a][:], mi[b][:])
                        if extra is not None:
                            c1_, c2_ = extra
                            if c1_ < 0:
                                nc.vector.tensor_sub(z[:], z[:], mi[-c1_][:])
                            else:
                                nc.vector.tensor_add(z[:], z[:], mi[c1_][:])
                            if c2_ < 0:
                                nc.vector.tensor_sub(z[:], z[:], mi[-c2_][:])
                            else:
                                nc.vector.tensor_add(z[:], z[:], mi[c2_][:])
                        gt = g_pool.tile([128, NFREE], F16,
                                         name=f"g_{gate}_{m}_{n}",
                                         tag=f"g{gate}", bufs=2)
                        nc.scalar.activation(
                            gt[:], z[:], act,
                            bias=bt_t[:, bi * MT + m: bi * MT + m + 1])
                        gates[(gate, n)] = gt

                go_sb = {}
                if m < 3:
                    # o-gate strassen for the pair (m, m+4), interleaved
                    # into the m-loop so its weight traffic spreads across
                    # the iteration instead of colliding with the early
                    # cb/xt DMAs (a standalone phase-O stalled 22us).
                    mi_o = {}
                    for s_ in (2, 5, 1, 3, 4, 6, 7):
                        wso = w_so[s_]
                        ps_s = p_pool.tile([128, NFREE], F32,
                                           name=f"ps_so{s_}_{m}", tag="ps")
                        for k in range(KS):
                            nc.tensor.matmul(
                                ps_s[:], wso[:, k, :], s_rhs(s_, k),
                                start=(k == 0), stop=(k == KS - 1))
                        t = g_pool.tile([128, NFREE], F16,
                                        name=f"mo_{s_}_{m}",
                                        tag=f"mi{s_}", bufs=1)
                        nc.scalar.copy(t[:], ps_s[:])
                        mi_o[s_] = t
                    for half, mm in ((0, m), (1, m + 4)):
                        for n in range(NT):
                            z = s_combine(mi_o, half, n)
                            go = e_pool.tile([128, NFREE], F16,
                                             name=f"go_{mm}_{n}", tag="go")
                            nc.scalar.activation(
                                go[:], z[:], SIG,
                                bias=bt_t[:, 4 * MT + mm: 4 * MT + mm + 1])
                            if half == 0:
                                go_sb[n] = go
                            else:
                                nc.sync.dma_start(
                                    gsp_d[mm][:, n * NFREE:(n + 1) * NFREE],
                                    go[:])

                # c-chain: independent of o, overlaps o's matmuls.  cl/cr
                # arrive fp16 and the fl*cl / fr*cr products overwrite them
                # in place (saves two fp32 scratch tags of SBUF).
                th_tiles = {}
                for n in range(NT):
                    sp = slice(m * 128, (m + 1) * 128)
                    sf = slice(n * NFREE, (n + 1) * NFREE)
                    cl_t = e_pool.tile([128, NFREE], F16,
                                       name=f"cl_{m}_{n}", tag="cl")
                    nc.sync.dma_start(cl_t[:], clt_d[sp, sf])
                    cr_t = e_pool.tile([128, NFREE], F16,
                                       name=f"cr_{m}_{n}", tag="cr")
                    nc.sync.dma_start(cr_t[:], crt_d[sp, sf])

                    iu = e_pool.tile([128, NFREE], F32,
                                     name=f"iu_{m}_{n}", tag="iu")
                    nc.vector.tensor_mul(iu[:], gates[("i", n)][:],
                                         gates[("u", n)][:])
                    nc.vector.tensor_mul(cl_t[:], gates[("fl", n)][:],
                                         cl_t[:])
                    nc.vector.tensor_mul(cr_t[:], gates[("fr", n)][:],
                                         cr_t[:])
                    # c accumulates in-place in iu
                    nc.vector.tensor_add(iu[:], iu[:], cl_t[:])
                    nc.vector.tensor_add(iu[:], iu[:], cr_t[:])
                    nc.sync.dma_start(ct_d[sp, sf], iu[:])

                    th = e_pool.tile([128, NFREE], F16,
                                     name=f"th_{m}_{n}", tag="th")
                    nc.scalar.activation(th[:], iu[:], TANH)
                    th_tiles[n] = th

                if not plain_o:
                    # o came from the in-loop strassen pair (m<3: SBUF;
                    # m>=4: DRAM spill reload)
                    for n in range(NT):
                        sp = slice(m * 128, (m + 1) * 128)
                        sf = slice(n * NFREE, (n + 1) * NFREE)
                        h_t = e_pool.tile([128, NFREE], F32,
                                          name=f"h_{m}_{n}", tag="h")
                        o_ap = go_sb[n][:] if m < 3 else gor[:, sf]
                        nc.vector.tensor_mul(h_t[:], o_ap, th_tiles[n][:])
                        nc.sync.dma_start(ht_d[sp, sf], h_t[:])
                    continue

                ps_o = gemm("o", m, w_o, n_outer=True)
                for n in range(NT):
                    sp = slice(m * 128, (m + 1) * 128)
                    sf = slice(n * NFREE, (n + 1) * NFREE)
                    # chunk the very last eviction's ACT/DVE so they
                    # pipeline on the kernel tail; keep a single DMA (a
                    # second dma_start costs ~0.6us of serialized sync-queue
                    # issue, more than the larger transfer)
                    last = (m == MT - 1 and n == NT - 1)
                    nch = 2 if last else 1
                    cw = NFREE // nch
                    go = e_pool.tile([128, NFREE], F16,
                                     name=f"g_o_{m}_{n}", tag="go")
                    h_t = e_pool.tile([128, NFREE], F32,
                                      name=f"h_{m}_{n}", tag="h")
                    for ci in range(nch):
                        cs = slice(ci * cw, (ci + 1) * cw)
                        nc.scalar.activation(
                            go[:, cs], ps_o[n][:, cs], SIG,
                            bias=bt_t[:, 4 * MT + m: 4 * MT + m + 1])
                        nc.vector.tensor_mul(h_t[:, cs], go[:, cs],
                                             th_tiles[n][:, cs])
                    nc.sync.dma_start(ht_d[sp, sf], h_t[:])

    nc.compile()
    return nc


def _get_program():
    if "nc" not in _CACHE:
        _CACHE["nc"] = _build_program()
    return _CACHE["nc"]


def _tile_weight(V, nk, mt=MT):
    """[nk*128, mt*128] -> [mt, 128, nk*128] with [m][kp, k*128+mc] = V[k*128+kp, m*128+mc]."""
    return np.ascontiguousarray(
        V.reshape(nk, 128, mt, 128)
         .transpose(2, 1, 0, 3)
         .reshape(mt, 128, nk * 128)
         .astype(np.float16))


def kernel(hl, cl, hr, cr, p,
           Wd, Wdl, Wdr, bd,
           Wf, Wfll, Wflr, Wfrl, Wfrr, bfl, bfr,
           Wo, Wol, Wor, bo,
           Wi, Wil, Wir, bi):
    global LAST_RESULTS
    f32 = np.float32
    hl, cl, hr, cr, p = (np.asarray(a, dtype=f32) for a in (hl, cl, hr, cr, p))
    ws = {k: np.asarray(v, dtype=f32) for k, v in dict(
        Wd=Wd, Wdl=Wdl, Wdr=Wdr, Wf=Wf, Wfll=Wfll, Wflr=Wflr, Wfrl=Wfrl,
        Wfrr=Wfrr, Wo=Wo, Wol=Wol, Wor=Wor, Wi=Wi, Wil=Wil, Wir=Wir).items()}

    # Wf{gate l/r}{child l/r}: f_left mixes hl via Wfll and hr via Wflr;
    # f_right mixes hl via Wfrl and hr via Wfrr.
    # i|u joint weight block [K=3072, M=2048] split into Strassen quadrants
    # (A11=[K1,i], A12=[K1,u], A21=[K2,i], A22=[K2,u]; K1 = first 1536 rows)
    # and combined on the host in fp32 before the fp16 cast.
    Wi_blk = np.concatenate([ws["Wd"].T, ws["Wdl"].T, ws["Wdr"].T], 0)
    Wu_blk = np.concatenate([ws["Wi"].T, ws["Wil"].T, ws["Wir"].T], 0)
    K1 = KS * 128
    A11, A21 = Wi_blk[:K1], Wi_blk[K1:]
    A12, A22 = Wu_blk[:K1], Wu_blk[K1:]
    s_w = {1: A11 + A22, 2: A12 + A22, 3: A11, 4: A22,
           5: A11 + A21, 6: A12 - A11, 7: A21 - A22}
    wt = {
        "o": _tile_weight(np.concatenate(
            [ws["Wo"].T, ws["Wol"].T, ws["Wor"].T], 0), KT),
        "pf": _tile_weight(np.ascontiguousarray(ws["Wf"].T), 8),
        "fl": _tile_weight(np.concatenate(
            [ws["Wfll"].T, ws["Wflr"].T], 0), 16),
        "fr": _tile_weight(np.concatenate(
            [ws["Wfrl"].T, ws["Wfrr"].T], 0), 16),
    }
    for s, Vw in s_w.items():
        wt[f"s{s}"] = _tile_weight(np.ascontiguousarray(Vw), KS)
    # o-gate strassen quadrants: M-half = 512 (4 m-tiles)
    Wo_blk = np.concatenate([ws["Wo"].T, ws["Wol"].T, ws["Wor"].T], 0)
    O11, O12 = Wo_blk[:K1, :512], Wo_blk[:K1, 512:]
    O21, O22 = Wo_blk[K1:, :512], Wo_blk[K1:, 512:]
    so_w = {1: O11 + O22, 2: O12 + O22, 3: O11, 4: O22,
            5: O11 + O21, 6: O12 - O11, 7: O21 - O22}
    so_t = {s: _tile_weight(np.ascontiguousarray(Vw), KS, mt=4)
            for s, Vw in so_w.items()}

    Bt = np.empty((128, 5 * MT), dtype=f32)
    for name, b_ in (("i", bd), ("fl", bfl), ("fr", bfr), ("u", bi), ("o", bo)):
        gi = BIAS_IDX[name]
        Bt[:, gi * MT:(gi + 1) * MT] = np.asarray(b_, dtype=f32).reshape(MT, 128).T

    X = np.concatenate([p, hl, hr], axis=1)    # [B, 3D]

    f16 = np.float16
    in_maps = []
    for r in range(NCORES):
        rows = slice(r * BL, (r + 1) * BL)
        XT = X[rows].T.astype(f16)                    # [3D, BL] fp16
        XTf = XT.astype(f32)
        N1 = NFREE
        B11, B12 = XTf[:K1, :N1], XTf[:K1, N1:]
        B21, B22 = XTf[K1:, :N1], XTf[K1:, N1:]
        cbs = {1: B11 + B22, 2: B12 - B22, 3: B21 - B11,
               4: B11 + B12, 5: B21 + B22}
        im = {
            "xt": np.ascontiguousarray(XT),
            "clt": np.ascontiguousarray(cl[rows].T.astype(f16)),
            "crt": np.ascontiguousarray(cr[rows].T.astype(f16)),
            "bt": Bt,
        }
        for j, cb in cbs.items():
            im[f"cb{j}"] = np.ascontiguousarray(
                cb.astype(f16).reshape(KS, 128, NFREE))
        for kind, arr in wt.items():
            im[f"w_{kind}"] = arr
        for s, arr in so_t.items():
            im[f"so{s}"] = arr
        in_maps.append(im)

    nc = _get_program()
    res = bass_utils.run_bass_kernel_spmd(nc, in_maps,
                                          core_ids=list(range(NCORES)))
    LAST_RESULTS = res

    h = np.empty((B, H), dtype=f32)
    c = np.empty((B, H), dtype=f32)
    for r in range(NCORES):
        rows = slice(r * BL, (r + 1) * BL)
        h[rows] = res.results[r]["ht"].T
        c[rows] = res.results[r]["ct"].T
    return (h, c)
